# revision 21
# baseline (speedup 1.0000x reference)
"""Multi-head attention forward on 8 TRN2 NeuronCores, data-parallel over batch.

Reference computation (per batch element b):
    qkv  = x @ qkv_w.T + qkv_b                     # [N, 3D]
    q, k = LN_headdim(q), LN_headdim(k)            # layernorm over head_dim=64
    S    = q @ k.T * hd^-0.5 ; A = softmax_j(S)    # per head
    out  = (A @ v) @ proj_w.T + proj_b             # [N, D]

v2 design (one batch element per core, no collectives), fully software-
pipelined so TensorE never waits for a phase boundary:
  - QKV is COLUMN-SLICED BY HEAD-PAIR: 6 pairs x 384 cols (q128|k128|v128,
    host-packed).  Pairs 0-1 prime the pipe; pair hp's 8 matmul groups ride
    inside head 2(hp-2)'s score stream, so the exp stream starts ~20us in
    and runs continuously to the end.  Group PSUM tiles ([P,384] f32, one
    bank) borrow the scores-tag rotation - PSUM stays within 8 banks
    (st [P,1024]x2 + av [P,512]x4).
  - k is NOT centered: against a fully-normalized q (sum_d q_n[d] = 0) the
    -mu_k term of k's layernorm vanishes in q_n.k; rstd_k folds into the
    exp's per-partition scale (scores^T has k-tokens on partitions), so k
    goes STRAIGHT from the QKV evacuation buffer into the block-transpose.
  - LN stats via one DVE bn_stats per (pair, tile); the per-pair rstd chain
    computes rstd = exp(-0.5 ln(var+eps)) on ScalarE - Ln/Exp/Identity all
    live in the SAME activation table as the softmax Exp, so the ACT table
    is never reloaded mid-stream (Sqrt would force a reload).
  - Scores computed TRANSPOSED: E = exp(scale_k * (k . q_n)) lands with
    k-tokens on partitions, directly the rhs of attn@v with V as lhsT.
  - q normalized into two zero-padded token-major buffers; plain 128x128
    block DMA transposes yield the K=128 zero-padded scores rhs.
  - Softmax denominators via 64 ones-columns in V (PSUM rows 64:128 hold
    the sums); normalize = SBUF copy + reciprocal_approx_fast + one VectorE
    multiply writing attnoutT.  (reciprocal_approx_fast must NOT read PSUM
    directly: it returns garbage on HW while passing in CoreSim.)
  - Heads software-pipelined 1:1 (scores of h interleave attn@v of h-1);
    projection computes outT = projwT.T @ attnoutT at the tail; ScalarE
    (idle after exps) applies the bias during PSUM evacuation; host flips.
"""

import os
import sys

import numpy as np

sys.path.insert(0, "/opt/trn_rl_repo")

from contextlib import ExitStack

import concourse.bass as bass
import concourse.tile as tile
from concourse import bacc, mybir
from concourse.bass_utils import run_bass_kernel_spmd

B, N, D = 8, 1024, 768
H, HD = 12, 64
NP = H // 2        # 6 head pairs
P = 128
NT = N // P        # 8 token tiles
DC = D // P        # 6 contraction subtiles
GC = 3 * P         # 384 qkv columns per pair group (q|k|v)
EPS = 1e-5
SCALE = HD ** -0.5  # 0.125
F32 = mybir.dt.float32
BF16 = mybir.dt.bfloat16


def _bcast_ap(ap_1d, parts):
    """View a 1-D DRAM AP as [parts, n] with partition stride 0 (broadcast)."""
    return bass.AP(
        tensor=ap_1d.tensor,
        offset=ap_1d.offset,
        ap=[[0, parts]] + list(ap_1d.ap),
    )


def _build_graph(apply_gn):
    nc = bacc.Bacc("TRN2", target_bir_lowering=False, debug=False, num_devices=B)

    x_d = nc.dram_tensor("x", [D, N], BF16, kind="ExternalInput").ap()
    # host-packed per-pair qkv weights: [D, pair*384] with 384 = q|k|v cols
    qkvw_d = nc.dram_tensor("qkv_wp", [D, NP * GC], BF16, kind="ExternalInput").ap()
    # host-packed biases: per-pair q|k (256 cols), per-head v (64)
    bqk_d = nc.dram_tensor("qkv_bqk", [NP * 2 * P], BF16, kind="ExternalInput").ap()
    bv_d = nc.dram_tensor("qkv_bv", [H * HD], BF16, kind="ExternalInput").ap()
    projw_d = nc.dram_tensor("proj_w", [D, D], BF16, kind="ExternalInput").ap()
    projb_d = nc.dram_tensor("proj_b", [D], F32, kind="ExternalInput").ap()
    gamma_d = nc.dram_tensor("qn_gamma", [HD], F32, kind="ExternalInput").ap()
    beta_d = nc.dram_tensor("qn_beta", [HD], F32, kind="ExternalInput").ap()
    # output is produced TRANSPOSED ([e, t]); the host flips it back
    out_d = nc.dram_tensor("out", [D, N], F32, kind="ExternalOutput").ap()

    with tile.TileContext(nc) as tc:
        _emit(tc, out_d, x_d, qkvw_d, bqk_d, bv_d, projw_d, projb_d,
              gamma_d, beta_d, apply_gn)

    nc.compile()
    return nc


def _emit(tc, out_d, x_d, qkvw_d, bqk_d, bv_d, projw_d, projb_d,
          gamma_d, beta_d, apply_gn):
    nc = tc.nc
    ctx = ExitStack()
    with ctx:
        const = ctx.enter_context(tc.tile_pool(name="const", bufs=1))
        wpool = ctx.enter_context(tc.tile_pool(name="wts", bufs=1))
        data = ctx.enter_context(tc.tile_pool(name="data", bufs=1))
        epool = ctx.enter_context(tc.tile_pool(name="escore", bufs=2))
        qkpool = ctx.enter_context(tc.tile_pool(name="qk", bufs=2))
        spool = ctx.enter_context(tc.tile_pool(name="stats", bufs=2))
        outp = ctx.enter_context(tc.tile_pool(name="outp", bufs=3))
        nrm = ctx.enter_context(tc.tile_pool(name="nrm", bufs=2))
        ps = ctx.enter_context(tc.tile_pool(name="ps", bufs=1, space="PSUM"))

        # ---- weight / input DMAs, interleaved so pair-0 work can start
        # as early as possible: x[dc] + pair-0 weights first ----
        xT = wpool.tile([P, DC, N], BF16)            # [d_in, dc, t]
        qkvwT = wpool.tile([P, DC, NP, GC], BF16)    # [d_in, dc, pair, 384]
        projwT = wpool.tile([P, DC, D], BF16)        # [o_in, oc, e]
        x_r = x_d.rearrange("(dc p) t -> p dc t", p=P)
        w_r = qkvw_d.rearrange("(dc p) x -> p dc x", p=P)
        # split DMAs so they round-robin over many queues (one 256KB chunk
        # on one queue is ~12us; the first group needs ALL x chunks).  Issue
        # the latency-critical ones from the otherwise-idle GpSimd sequencer:
        # SP pays ~1.2us of DGE time per issue, Pool ~25ns.
        for dc in range(DC):
            for th in range(2):
                nc.gpsimd.dma_start(xT[:, dc, th * 512:(th + 1) * 512],
                                    x_r[:, dc, th * 512:(th + 1) * 512])
            nc.gpsimd.dma_start(qkvwT[:, dc, 0, :], w_r[:, dc, 0:GC])
        for dc in range(DC):
            nc.gpsimd.dma_start(qkvwT[:, dc, 1, :], w_r[:, dc, GC:2 * GC])
        for hp in range(2, 4):
            for dc in range(DC):
                nc.gpsimd.dma_start(
                    qkvwT[:, dc, hp, :], w_r[:, dc, hp * GC:(hp + 1) * GC]
                )

        # broadcast constants (after the first-needed matmul operands)
        bqk_bc = const.tile([P, NP, 2 * P], BF16)
        nc.gpsimd.dma_start(bqk_bc[:], _bcast_ap(bqk_d, P))
        bv_bc = const.tile([P, H, HD], BF16)
        nc.gpsimd.dma_start(bv_bc[:], _bcast_ap(bv_d, P))
        projb_col = const.tile([P, DC], F32)
        nc.gpsimd.dma_start(projb_col[:], projb_d.rearrange("(et p) -> p et", p=P))
        if apply_gn:
            gamma_bc = const.tile([P, HD], F32)
            nc.gpsimd.dma_start(gamma_bc[:], _bcast_ap(gamma_d, P))
            beta_bc = const.tile([P, HD], F32)
            nc.gpsimd.dma_start(beta_bc[:], _bcast_ap(beta_d, P))

        for hp in range(4, NP):
            for dc in range(DC):
                nc.gpsimd.dma_start(
                    qkvwT[:, dc, hp, :], w_r[:, dc, hp * GC:(hp + 1) * GC]
                )
        for dc in range(DC):
            nc.gpsimd.dma_start(
                projwT[:, dc, :],
                projw_d.rearrange("(dc p) e -> p dc e", p=P)[:, dc, :],
            )

        # ---- persistent SBUF data tiles ----
        # qkv evacuations (per pair, double-buffered by pair parity); q and k
        # in separate tiles so the k block-transpose source is 2D-contiguous.
        # cols 0:64 = even head, 64:128 = odd head
        q_ev = data.tile([P, 2, NT, P], BF16)
        k_ev = data.tile([P, 2, NT, P], BF16)
        # q normalized, token-major, zero-padded halves (for DMA transpose)
        qnp0 = data.tile([P, 2, NT, P], BF16)   # cols 0:64 = q even head
        qnp1 = data.tile([P, 2, NT, P], BF16)   # cols 64:128 = q odd head
        # v with 64 ones-columns: attn@v psum rows 64:128 = softmax denoms.
        # memsets split per token-tile so the first QKV evacuations don't
        # serialize behind one long DVE memset.
        vext = data.tile([P, NT, H, 2 * HD], BF16)
        for tt in range(NT):
            nc.vector.memset(vext[:, tt, :, HD:2 * HD], 1.0)
        for pb in range(2):
            nc.vector.memset(qnp0[:, pb, :, HD:2 * HD], 0.0)
            nc.vector.memset(qnp1[:, pb, :, 0:HD], 0.0)
        # attnoutT [o_in, oc, t] written by the normalize step
        attnoutT = data.tile([P, DC, N], BF16)
        # 0.125 * rstd_k per (token-tile, head): per-partition exp scales
        rks = data.tile([P, NT, H], F32)
        # bn_stats output per pair: [P, parity, tt, 4 groups, 6]
        bnout = data.tile([P, 2, NT, 4, 6], F32)
        # per-pair q-norm params [P, parity, tt, grp] (0=q_even 1=q_odd)
        rstdq = data.tile([P, 2, NT, 2], F32)
        m2q = data.tile([P, 2, NT, 2], F32)
        if apply_gn:
            rstdk = data.tile([P, 2, NT, 2], F32)
            m2k = data.tile([P, 2, NT, 2], F32)

        # ---------------- emission helpers ----------------
        def st_tile():
            return ps.tile([P, N], F32, tag="st", name="ps_st", bufs=2)

        def av_tile():
            return ps.tile([P, 512], F32, tag="av", name="ps_av", bufs=4)

        def emit_group(hp, tt):
            """One QKV matmul group: psum[:, 0:384] = x_tt @ w_pair_hp,
            then evacuations (q|k to qk_ev, v to vext) and bn_stats."""
            pg = st_tile()
            for dc in range(DC):
                nc.tensor.matmul(
                    pg[:, 0:GC],
                    lhsT=xT[:, dc, tt * P:(tt + 1) * P],
                    rhs=qkvwT[:, dc, hp, :],
                    start=(dc == 0),
                    stop=(dc == DC - 1),
                )
            pb = hp % 2
            nc.vector.tensor_tensor(
                q_ev[:, pb, tt, :], pg[:, 0:P], bqk_bc[:, hp, 0:P],
                op=mybir.AluOpType.add,
            )
            nc.vector.tensor_tensor(
                k_ev[:, pb, tt, :], pg[:, P:2 * P], bqk_bc[:, hp, P:2 * P],
                op=mybir.AluOpType.add,
            )
            nc.vector.tensor_tensor(
                vext[:, tt, 2 * hp:2 * hp + 2, 0:HD],
                pg[:, 2 * P:3 * P].rearrange("p (s h) -> p s h", h=HD),
                bv_bc[:, 2 * hp:2 * hp + 2, :],
                op=mybir.AluOpType.add,
            )
            # HW restriction: one bn_stats = one 6-element output group
            for par in range(2):
                nc.vector.bn_stats(
                    bnout[:, pb, tt, par],
                    q_ev[:, pb, tt, par * HD:(par + 1) * HD],
                )
                nc.vector.bn_stats(
                    bnout[:, pb, tt, 2 + par],
                    k_ev[:, pb, tt, par * HD:(par + 1) * HD],
                )

        def emit_pair_stats(hp):
            """Per-pair rstd/m2 chain from bnout; fills rstdq/m2q/rks.
            bn_stats gives per group: [cnt_e, mean_e, M2_e, cnt_o, mean_o,
            M2_o] over even/odd elements.  mean = (me+mo)/2 and
            var = (M2e+M2o)/64 + ((me-mo)/2)^2."""
            pb = hp % 2
            me = bnout[:, pb, :, :, 1]    # [P, NT, 4]
            mo = bnout[:, pb, :, :, 4]
            M2e = bnout[:, pb, :, :, 2]
            M2o = bnout[:, pb, :, :, 5]
            a = spool.tile([P, NT, 4], F32, tag="a", name="sa")
            d = spool.tile([P, NT, 4], F32, tag="d", name="sd")
            var = spool.tile([P, NT, 4], F32, tag="var", name="svar")
            mu = spool.tile([P, NT, 4], F32, tag="mu", name="smu")
            rst = spool.tile([P, NT, 4], F32, tag="rst", name="srst")
            nc.vector.tensor_tensor(a, M2e, M2o, op=mybir.AluOpType.add)
            nc.vector.tensor_tensor(d, me, mo, op=mybir.AluOpType.subtract)
            nc.vector.tensor_tensor(d, d, d, op=mybir.AluOpType.mult)
            nc.vector.tensor_scalar(a, a, 1.0 / HD, EPS,
                                    op0=mybir.AluOpType.mult,
                                    op1=mybir.AluOpType.add)
            nc.vector.tensor_scalar(d, d, 0.25, 0.0,
                                    op0=mybir.AluOpType.mult,
                                    op1=mybir.AluOpType.add)
            nc.vector.tensor_tensor(var, a, d, op=mybir.AluOpType.add)
            # rstd = exp(-0.5 * ln(var+eps)); Ln/Exp share the act table
            nc.scalar.activation(a, var, mybir.ActivationFunctionType.Ln)
            nc.scalar.activation(rst, a, mybir.ActivationFunctionType.Exp,
                                 scale=-0.5)
            # m2 = -mean * rstd  (qnorm per-partition bias)
            nc.vector.tensor_tensor(mu, me, mo, op=mybir.AluOpType.add)
            nc.vector.tensor_tensor(mu, mu, rst, op=mybir.AluOpType.mult)
            nc.vector.tensor_scalar(mu, mu, -0.5, 0.0,
                                    op0=mybir.AluOpType.mult,
                                    op1=mybir.AluOpType.add)
            nc.vector.tensor_copy(rstdq[:, pb], rst[:, :, 0:2])
            nc.vector.tensor_copy(m2q[:, pb], mu[:, :, 0:2])
            if not apply_gn:
                # k rstd -> exp scale table (0.125 * rstd_k)
                nc.vector.tensor_scalar(
                    rks[:, :, 2 * hp:2 * hp + 2], rst[:, :, 2:4], SCALE, 0.0,
                    op0=mybir.AluOpType.mult, op1=mybir.AluOpType.add)
            else:
                nc.vector.tensor_copy(rstdk[:, pb], rst[:, :, 2:4])
                nc.vector.tensor_copy(m2k[:, pb], mu[:, :, 2:4])

        def emit_pair_norms(hp):
            """q normalize into qnp0/qnp1 (+ for gn: full k LN in place)."""
            pb = hp % 2
            for tt in range(NT):
                for par in range(2):
                    dst = (qnp1[:, pb, tt, HD:2 * HD] if par
                           else qnp0[:, pb, tt, 0:HD])
                    nc.vector.tensor_scalar(
                        dst, q_ev[:, pb, tt, par * HD:(par + 1) * HD],
                        rstdq[:, pb, tt, par:par + 1],
                        m2q[:, pb, tt, par:par + 1],
                        op0=mybir.AluOpType.mult, op1=mybir.AluOpType.add)
                    if apply_gn:
                        nc.gpsimd.tensor_tensor(dst, dst, gamma_bc[:, 0:HD],
                                                op=mybir.AluOpType.mult)
                        nc.gpsimd.tensor_tensor(dst, dst, beta_bc[:, 0:HD],
                                                op=mybir.AluOpType.add)
                        kd = k_ev[:, pb, tt, par * HD:(par + 1) * HD]
                        nc.vector.tensor_scalar(
                            kd, kd,
                            rstdk[:, pb, tt, par:par + 1],
                            m2k[:, pb, tt, par:par + 1],
                            op0=mybir.AluOpType.mult, op1=mybir.AluOpType.add)
                        nc.gpsimd.tensor_tensor(kd, kd, gamma_bc[:, 0:HD],
                                                op=mybir.AluOpType.mult)
                        nc.gpsimd.tensor_tensor(kd, kd, beta_bc[:, 0:HD],
                                                op=mybir.AluOpType.add)

        def emit_pair_transposes(hp):
            # split each transpose into 2-block chunks so the descriptor
            # streams round-robin across DMA queues (a whole [P, 8x128]
            # transpose on one queue takes ~10us)
            pb = hp % 2
            kkT = qkpool.tile([P, N], BF16, tag="kkT", name="kkT", bufs=3)
            qp0 = qkpool.tile([P, N], BF16, tag="qp0", name="qp0", bufs=3)
            qp1 = qkpool.tile([P, N], BF16, tag="qp1", name="qp1", bufs=3)
            for dst, src in ((kkT, k_ev[:, pb]), (qp0, qnp0[:, pb]),
                             (qp1, qnp1[:, pb])):
                dr = dst.rearrange("p (b t) -> p b t", t=P)
                for c in range(0, NT, 2):
                    nc.sync.dma_start_transpose(dr[:, c:c + 2], src[:, c:c + 2])
            return kkT, qp0, qp1

        def emit_normalize(h, pa0, pa1):
            for ic, pa in ((0, pa0), (1, pa1)):
                rcp_t = nrm.tile([HD, 512], F32, tag="rcp_t", name="rcp_t")
                s_sb = nrm.tile([HD, 512], F32, tag="s_sb", name="s_sb")
                nc.vector.tensor_copy(s_sb[:], pa[HD:2 * HD, :])
                nc.vector.reciprocal_approx_fast(rcp_t[:], s_sb[:])
                nc.vector.tensor_tensor(
                    attnoutT[(h % 2) * HD:(h % 2 + 1) * HD, h // 2,
                             ic * 512:(ic + 1) * 512],
                    pa[0:HD, :],
                    rcp_t[:],
                    op=mybir.AluOpType.mult,
                )

        def emit_head(h, kkT, qp0, qp1, prev, gsrc):
            """Scores+exp for head h, 1:1 interleaved with the attn@v of
            head h-1 (prev), plus one QKV group of pair gsrc per jt slot."""
            qT = qp0 if h % 2 == 0 else qp1
            E = epool.tile([P, NT, N], BF16, tag="E", name="E")
            if prev is not None:
                hprev, Eprev = prev
                pa0 = av_tile()
                pa1 = av_tile()
            for jt in range(NT):
                pst = st_tile()
                for ic in range(2):
                    nc.tensor.matmul(
                        pst[:, ic * 512:(ic + 1) * 512],
                        lhsT=kkT[:, jt * P:(jt + 1) * P],
                        rhs=qT[:, ic * 512:(ic + 1) * 512],
                        start=True,
                        stop=True,
                    )
                if apply_gn:
                    nc.scalar.activation(
                        E[:, jt, :], pst,
                        mybir.ActivationFunctionType.Exp, scale=SCALE)
                else:
                    nc.scalar.activation(
                        E[:, jt, :], pst,
                        mybir.ActivationFunctionType.Exp,
                        scale=rks[:, jt, h:h + 1])
                if prev is not None:
                    nc.tensor.matmul(
                        pa0, lhsT=vext[:, jt, hprev, :],
                        rhs=Eprev[:, jt, 0:512],
                        start=(jt == 0), stop=(jt == NT - 1),
                    )
                    nc.tensor.matmul(
                        pa1, lhsT=vext[:, jt, hprev, :],
                        rhs=Eprev[:, jt, 512:1024],
                        start=(jt == 0), stop=(jt == NT - 1),
                    )
                if gsrc is not None and jt % 2 == 1:
                    emit_group(gsrc, (h % 2) * 4 + jt // 2)
            if prev is not None:
                emit_normalize(hprev, pa0, pa1)
            return E

        def emit_av_tail(h, E):
            pa0 = av_tile()
            pa1 = av_tile()
            for jt in range(NT):
                nc.tensor.matmul(
                    pa0, lhsT=vext[:, jt, h, :], rhs=E[:, jt, 0:512],
                    start=(jt == 0), stop=(jt == NT - 1),
                )
                nc.tensor.matmul(
                    pa1, lhsT=vext[:, jt, h, :], rhs=E[:, jt, 512:1024],
                    start=(jt == 0), stop=(jt == NT - 1),
                )
            emit_normalize(h, pa0, pa1)

        # ---------------- the pipeline ----------------
        # prime: pairs 0 and 1
        tps = {}
        for tt in range(NT):
            emit_group(0, tt)
        emit_pair_stats(0)
        emit_pair_norms(0)
        tps[0] = emit_pair_transposes(0)
        for tt in range(NT):
            emit_group(1, tt)
        emit_pair_stats(1)
        emit_pair_norms(1)
        tps[1] = emit_pair_transposes(1)

        prev = None
        for h in range(H):
            hp, hh = divmod(h, 2)
            # pair hp+2's groups ride this pair's heads, 4 per head at odd jt
            gsrc = hp + 2 if hp + 2 < NP else None
            E = emit_head(h, *tps[hp], prev, gsrc)
            if hh == 1 and gsrc is not None:
                emit_pair_stats(gsrc)
                emit_pair_norms(gsrc)
                tps[gsrc] = emit_pair_transposes(gsrc)
                del tps[hp]
            prev = (h, E)
        emit_av_tail(*prev)

        # ---- output projection: outT[e, t] = projwT.T @ attnoutT ----
        for et in range(DC):
            ps0 = av_tile()
            ps1 = av_tile()
            for oc in range(DC):
                for half, pp in ((0, ps0), (1, ps1)):
                    nc.tensor.matmul(
                        pp,
                        lhsT=projwT[:, oc, et * P:(et + 1) * P],
                        rhs=attnoutT[:, oc, half * 512:(half + 1) * 512],
                        start=(oc == 0),
                        stop=(oc == DC - 1),
                    )
            for half, pp in ((0, ps0), (1, ps1)):
                ot = outp.tile([P, 512], F32, tag="outt", name="ot")
                nc.scalar.activation(
                    ot[:], pp, mybir.ActivationFunctionType.Identity,
                    bias=projb_col[:, et:et + 1],
                )
                # split across DMA queues: one 256KB f32 chunk on one queue
                # is ~12us of tail latency
                for c in range(4):
                    nc.gpsimd.dma_start(
                        out_d[et * P:(et + 1) * P,
                              half * 512 + c * P:half * 512 + (c + 1) * P],
                        ot[:, c * P:(c + 1) * P],
                    )


_NC_CACHE = {}


def _get_nc(apply_gn=True):
    if apply_gn not in _NC_CACHE:
        _NC_CACHE[apply_gn] = _build_graph(apply_gn)
    return _NC_CACHE[apply_gn]


def make_in_maps(x, qkv_w, qkv_b, proj_w, proj_b, qn_gamma, qn_beta):
    """Host-side layout prep: transpose + bf16-cast x / weights; pack qkv
    weights and biases by head pair (q128|k128|v128 columns per pair)."""
    import ml_dtypes
    bf = ml_dtypes.bfloat16
    x = np.asarray(x, np.float32)
    qkv_w32 = np.asarray(qkv_w, np.float32)
    qkv_b32 = np.asarray(qkv_b, np.float32)
    wT = qkv_w32.T  # [D, 3D]: rows of qkv_w = out dims q|k|v
    pair_w = np.concatenate(
        [
            np.concatenate(
                [
                    wT[:, P * hp:P * (hp + 1)],
                    wT[:, D + P * hp:D + P * (hp + 1)],
                    wT[:, 2 * D + P * hp:2 * D + P * (hp + 1)],
                ],
                axis=1,
            )
            for hp in range(NP)
        ],
        axis=1,
    )  # [D, 6*384]
    bqk = np.concatenate(
        [
            np.concatenate(
                [qkv_b32[P * hp:P * (hp + 1)],
                 qkv_b32[D + P * hp:D + P * (hp + 1)]]
            )
            for hp in range(NP)
        ]
    )  # [6*256]
    bv = qkv_b32[2 * D:]  # [768] per-head v bias
    shared = {
        "qkv_wp": np.ascontiguousarray(pair_w.astype(bf)),
        "qkv_bqk": np.ascontiguousarray(bqk.astype(bf)),
        "qkv_bv": np.ascontiguousarray(bv.astype(bf)),
        "proj_w": np.ascontiguousarray(np.asarray(proj_w, np.float32).T.astype(bf)),
        "proj_b": np.ascontiguousarray(proj_b, np.float32),
        "qn_gamma": np.ascontiguousarray(qn_gamma, np.float32),
        "qn_beta": np.ascontiguousarray(qn_beta, np.float32),
    }
    return [
        {**shared, "x": np.ascontiguousarray(x[i].T.astype(bf))} for i in range(B)
    ]


def extract_output(res):
    return np.stack(
        [np.ascontiguousarray(res.results[i]["out"].T) for i in range(B)], axis=0
    )


def kernel(x, qkv_w, qkv_b, proj_w, proj_b, qn_gamma, qn_beta):
    qn_gamma = np.ascontiguousarray(qn_gamma, np.float32)
    qn_beta = np.ascontiguousarray(qn_beta, np.float32)
    apply_gn = not (np.all(qn_gamma == 1.0) and np.all(qn_beta == 0.0))
    nc = _get_nc(apply_gn)
    in_maps = make_in_maps(x, qkv_w, qkv_b, proj_w, proj_b, qn_gamma, qn_beta)
    res = run_bass_kernel_spmd(nc, in_maps, core_ids=list(range(B)))
    return extract_output(res)


# revision 26
# speedup vs baseline: 1.1619x; 1.1619x over previous
"""Multi-head attention forward on 8 TRN2 NeuronCores, data-parallel over batch.

Reference computation (per batch element b):
    qkv  = x @ qkv_w.T + qkv_b                     # [N, 3D]
    q, k = LN_headdim(q), LN_headdim(k)            # layernorm over head_dim=64
    S    = q @ k.T * hd^-0.5 ; A = softmax_j(S)    # per head
    out  = (A @ v) @ proj_w.T + proj_b             # [N, D]

v2 design (one batch element per core, no collectives), fully software-
pipelined so TensorE never waits for a phase boundary:
  - QKV is COLUMN-SLICED BY HEAD-PAIR: 6 pairs x 384 cols (q128|k128|v128,
    host-packed).  Pairs 0-1 prime the pipe; pair hp's 8 matmul groups ride
    inside head 2(hp-2)'s score stream, so the exp stream starts ~20us in
    and runs continuously to the end.  Group PSUM tiles ([P,384] f32, one
    bank) borrow the scores-tag rotation - PSUM stays within 8 banks
    (st [P,1024]x2 + av [P,512]x4).
  - k is NOT centered: against a fully-normalized q (sum_d q_n[d] = 0) the
    -mu_k term of k's layernorm vanishes in q_n.k; rstd_k folds into the
    exp's per-partition scale (scores^T has k-tokens on partitions), so k
    goes STRAIGHT from the QKV evacuation buffer into the block-transpose.
  - LN stats via one DVE bn_stats per (pair, tile); the per-pair rstd chain
    computes rstd = exp(-0.5 ln(var+eps)) on ScalarE - Ln/Exp/Identity all
    live in the SAME activation table as the softmax Exp, so the ACT table
    is never reloaded mid-stream (Sqrt would force a reload).
  - Scores computed TRANSPOSED: E = exp(scale_k * (k . q_n)) lands with
    k-tokens on partitions, directly the rhs of attn@v with V as lhsT.
  - q normalized into two zero-padded token-major buffers; plain 128x128
    block DMA transposes yield the K=128 zero-padded scores rhs.
  - Softmax denominators via 64 ones-columns in V (PSUM rows 64:128 hold
    the sums); normalize = SBUF copy + reciprocal_approx_fast + one VectorE
    multiply writing attnoutT.  (reciprocal_approx_fast must NOT read PSUM
    directly: it returns garbage on HW while passing in CoreSim.)
  - Heads software-pipelined 1:1 (scores of h interleave attn@v of h-1);
    projection computes outT = projwT.T @ attnoutT at the tail; ScalarE
    (idle after exps) applies the bias during PSUM evacuation; host flips.
"""

import os
import sys

import numpy as np

sys.path.insert(0, "/opt/trn_rl_repo")

from contextlib import ExitStack

import concourse.bass as bass
import concourse.tile as tile
from concourse import bacc, mybir
from concourse.bass_utils import run_bass_kernel_spmd

B, N, D = 8, 1024, 768
H, HD = 12, 64
NP = H // 2        # 6 head pairs
P = 128
NT = N // P        # 8 token tiles
DC = D // P        # 6 contraction subtiles
GC = 3 * P         # 384 qkv columns per pair group (q|k|v)
EPS = 1e-5
SCALE = HD ** -0.5  # 0.125
F32 = mybir.dt.float32
BF16 = mybir.dt.bfloat16


def _bcast_ap(ap_1d, parts):
    """View a 1-D DRAM AP as [parts, n] with partition stride 0 (broadcast)."""
    return bass.AP(
        tensor=ap_1d.tensor,
        offset=ap_1d.offset,
        ap=[[0, parts]] + list(ap_1d.ap),
    )


def _build_graph(apply_gn):
    nc = bacc.Bacc("TRN2", target_bir_lowering=False, debug=False, num_devices=B)

    x_d = nc.dram_tensor("x", [D, N], BF16, kind="ExternalInput").ap()
    # host-packed per-pair qkv weights: [D, pair*384] with 384 = q|k|v cols
    qkvw_d = nc.dram_tensor("qkv_wp", [D, NP * GC], BF16, kind="ExternalInput").ap()
    # host-packed biases: per-pair q|k (256 cols), per-head v (64)
    bqk_d = nc.dram_tensor("qkv_bqk", [NP * 2 * P], BF16, kind="ExternalInput").ap()
    bv_d = nc.dram_tensor("qkv_bv", [H * HD], BF16, kind="ExternalInput").ap()
    projw_d = nc.dram_tensor("proj_w", [D, D], BF16, kind="ExternalInput").ap()
    projb_d = nc.dram_tensor("proj_b", [D], F32, kind="ExternalInput").ap()
    gamma_d = nc.dram_tensor("qn_gamma", [HD], F32, kind="ExternalInput").ap()
    beta_d = nc.dram_tensor("qn_beta", [HD], F32, kind="ExternalInput").ap()
    # output is produced TRANSPOSED ([e, t]); the host flips it back
    out_d = nc.dram_tensor("out", [D, N], F32, kind="ExternalOutput").ap()

    with tile.TileContext(nc) as tc:
        _emit(tc, out_d, x_d, qkvw_d, bqk_d, bv_d, projw_d, projb_d,
              gamma_d, beta_d, apply_gn)

    nc.compile()
    return nc


def _emit(tc, out_d, x_d, qkvw_d, bqk_d, bv_d, projw_d, projb_d,
          gamma_d, beta_d, apply_gn):
    nc = tc.nc
    ctx = ExitStack()
    with ctx:
        const = ctx.enter_context(tc.tile_pool(name="const", bufs=1))
        wpool = ctx.enter_context(tc.tile_pool(name="wts", bufs=1))
        data = ctx.enter_context(tc.tile_pool(name="data", bufs=1))
        epool = ctx.enter_context(tc.tile_pool(name="escore", bufs=2))
        qkpool = ctx.enter_context(tc.tile_pool(name="qk", bufs=2))
        spool = ctx.enter_context(tc.tile_pool(name="stats", bufs=2))
        outp = ctx.enter_context(tc.tile_pool(name="outp", bufs=3))
        nrm = ctx.enter_context(tc.tile_pool(name="nrm", bufs=2))
        ps = ctx.enter_context(tc.tile_pool(name="ps", bufs=1, space="PSUM"))

        # ---- weight / input DMAs, interleaved so pair-0 work can start
        # as early as possible: x[dc] + pair-0 weights first ----
        xT = wpool.tile([P, DC, N], BF16)            # [d_in, dc, t]
        qkvwT = wpool.tile([P, DC, NP, GC], BF16)    # [d_in, dc, pair, 384]
        projwT = wpool.tile([P, DC, D], BF16)        # [o_in, oc, e]
        x_r = x_d.rearrange("(dc p) t -> p dc t", p=P)
        w_r = qkvw_d.rearrange("(dc p) x -> p dc x", p=P)
        # Every dma_start costs ~0.6-1.2us of DGE time on its ISSUING
        # sequencer, so keep splits coarse and spread the latency-critical
        # issues across the engines that are idle at that point (SP + DVE +
        # ACT before the exp stream starts; GpSimd for everything
        # non-critical).
        for dc in range(DC):
            nc.sync.dma_start(xT[:, dc, 0:512], x_r[:, dc, 0:512])
            nc.scalar.dma_start(xT[:, dc, 512:1024], x_r[:, dc, 512:1024])
            nc.scalar.dma_start(qkvwT[:, dc, 0, :], w_r[:, dc, 0:GC])
        for dc in range(DC):
            nc.sync.dma_start(qkvwT[:, dc, 1, :], w_r[:, dc, GC:2 * GC])
        for hp in range(2, 4):
            for dc in range(DC):
                nc.gpsimd.dma_start(
                    qkvwT[:, dc, hp, :], w_r[:, dc, hp * GC:(hp + 1) * GC]
                )

        # broadcast constants (after the first-needed matmul operands)
        bqk_bc = const.tile([P, NP, 2 * P], BF16)
        nc.gpsimd.dma_start(bqk_bc[:], _bcast_ap(bqk_d, P))
        bv_bc = const.tile([P, H, HD], BF16)
        nc.gpsimd.dma_start(bv_bc[:], _bcast_ap(bv_d, P))
        projb_col = const.tile([P, DC], F32)
        nc.gpsimd.dma_start(projb_col[:], projb_d.rearrange("(et p) -> p et", p=P))
        if apply_gn:
            gamma_bc = const.tile([P, HD], F32)
            nc.gpsimd.dma_start(gamma_bc[:], _bcast_ap(gamma_d, P))
            beta_bc = const.tile([P, HD], F32)
            nc.gpsimd.dma_start(beta_bc[:], _bcast_ap(beta_d, P))

        for hp in range(4, NP):
            for dc in range(DC):
                nc.gpsimd.dma_start(
                    qkvwT[:, dc, hp, :], w_r[:, dc, hp * GC:(hp + 1) * GC]
                )
        for dc in range(DC):
            nc.gpsimd.dma_start(
                projwT[:, dc, :],
                projw_d.rearrange("(dc p) e -> p dc e", p=P)[:, dc, :],
            )

        # ---- persistent SBUF data tiles ----
        # qkv evacuations (per pair, double-buffered by pair parity); q and k
        # in separate tiles so the k block-transpose source is 2D-contiguous.
        # cols 0:64 = even head, 64:128 = odd head
        q_ev = data.tile([P, 2, NT, P], BF16)
        k_ev = data.tile([P, 2, NT, P], BF16)
        # q normalized, token-major, zero-padded halves (for DMA transpose)
        qnp0 = data.tile([P, 2, NT, P], BF16)   # cols 0:64 = q even head
        qnp1 = data.tile([P, 2, NT, P], BF16)   # cols 64:128 = q odd head
        # v with 64 ones-columns: attn@v psum rows 64:128 = softmax denoms.
        # memsets split per token-tile so the first QKV evacuations don't
        # serialize behind one long DVE memset.
        vext = data.tile([P, NT, H, 2 * HD], BF16)
        for tt in range(NT):
            nc.vector.memset(vext[:, tt, :, HD:2 * HD], 1.0)
        for pb in range(2):
            nc.vector.memset(qnp0[:, pb, :, HD:2 * HD], 0.0)
            nc.vector.memset(qnp1[:, pb, :, 0:HD], 0.0)
        # attnoutT [o_in, oc, t] written by the normalize step
        attnoutT = data.tile([P, DC, N], BF16)
        # 0.125 * rstd_k per (token-tile, head): per-partition exp scales
        rks = data.tile([P, NT, H], F32)
        # bn_stats output per pair: [P, parity, tt, 4 groups, 6]
        bnout = data.tile([P, 2, NT, 4, 6], F32)
        # per-pair q-norm params [P, parity, tt, grp] (0=q_even 1=q_odd)
        rstdq = data.tile([P, 2, NT, 2], F32)
        m2q = data.tile([P, 2, NT, 2], F32)
        if apply_gn:
            rstdk = data.tile([P, 2, NT, 2], F32)
            m2k = data.tile([P, 2, NT, 2], F32)

        # ---------------- emission helpers ----------------
        def st_tile():
            return ps.tile([P, N], F32, tag="st", name="ps_st", bufs=2)

        def av_tile():
            return ps.tile([P, 512], F32, tag="av", name="ps_av", bufs=4)

        def emit_group(hp, tt):
            """One QKV matmul group: psum[:, 0:384] = x_tt @ w_pair_hp,
            then evacuations (q|k to qk_ev, v to vext) and bn_stats."""
            pg = st_tile()
            for dc in range(DC):
                nc.tensor.matmul(
                    pg[:, 0:GC],
                    lhsT=xT[:, dc, tt * P:(tt + 1) * P],
                    rhs=qkvwT[:, dc, hp, :],
                    start=(dc == 0),
                    stop=(dc == DC - 1),
                )
            pb = hp % 2
            nc.vector.tensor_tensor(
                q_ev[:, pb, tt, :], pg[:, 0:P], bqk_bc[:, hp, 0:P],
                op=mybir.AluOpType.add,
            )
            nc.vector.tensor_tensor(
                k_ev[:, pb, tt, :], pg[:, P:2 * P], bqk_bc[:, hp, P:2 * P],
                op=mybir.AluOpType.add,
            )
            nc.vector.tensor_tensor(
                vext[:, tt, 2 * hp:2 * hp + 2, 0:HD],
                pg[:, 2 * P:3 * P].rearrange("p (s h) -> p s h", h=HD),
                bv_bc[:, 2 * hp:2 * hp + 2, :],
                op=mybir.AluOpType.add,
            )
            # HW restriction: one bn_stats = one 6-element output group
            for par in range(2):
                nc.vector.bn_stats(
                    bnout[:, pb, tt, par],
                    q_ev[:, pb, tt, par * HD:(par + 1) * HD],
                )
                nc.vector.bn_stats(
                    bnout[:, pb, tt, 2 + par],
                    k_ev[:, pb, tt, par * HD:(par + 1) * HD],
                )

        def emit_pair_stats(hp):
            """Per-pair rstd/m2 chain from bnout; fills rstdq/m2q/rks.
            bn_stats gives per group: [cnt_e, mean_e, M2_e, cnt_o, mean_o,
            M2_o] over even/odd elements.  mean = (me+mo)/2 and
            var = (M2e+M2o)/64 + ((me-mo)/2)^2."""
            pb = hp % 2
            me = bnout[:, pb, :, :, 1]    # [P, NT, 4]
            mo = bnout[:, pb, :, :, 4]
            M2e = bnout[:, pb, :, :, 2]
            M2o = bnout[:, pb, :, :, 5]
            a = spool.tile([P, NT, 4], F32, tag="a", name="sa")
            d = spool.tile([P, NT, 4], F32, tag="d", name="sd")
            var = spool.tile([P, NT, 4], F32, tag="var", name="svar")
            mu = spool.tile([P, NT, 4], F32, tag="mu", name="smu")
            rst = spool.tile([P, NT, 4], F32, tag="rst", name="srst")
            nc.vector.tensor_tensor(a, M2e, M2o, op=mybir.AluOpType.add)
            nc.vector.tensor_tensor(d, me, mo, op=mybir.AluOpType.subtract)
            nc.vector.tensor_tensor(d, d, d, op=mybir.AluOpType.mult)
            nc.vector.tensor_scalar(a, a, 1.0 / HD, EPS,
                                    op0=mybir.AluOpType.mult,
                                    op1=mybir.AluOpType.add)
            nc.vector.tensor_scalar(d, d, 0.25, 0.0,
                                    op0=mybir.AluOpType.mult,
                                    op1=mybir.AluOpType.add)
            nc.vector.tensor_tensor(var, a, d, op=mybir.AluOpType.add)
            # rstd = exp(-0.5 * ln(var+eps)); Ln/Exp share the act table
            nc.scalar.activation(a, var, mybir.ActivationFunctionType.Ln)
            nc.scalar.activation(rst, a, mybir.ActivationFunctionType.Exp,
                                 scale=-0.5)
            # m2 = -mean * rstd  (qnorm per-partition bias)
            nc.vector.tensor_tensor(mu, me, mo, op=mybir.AluOpType.add)
            nc.vector.tensor_tensor(mu, mu, rst, op=mybir.AluOpType.mult)
            nc.vector.tensor_scalar(mu, mu, -0.5, 0.0,
                                    op0=mybir.AluOpType.mult,
                                    op1=mybir.AluOpType.add)
            nc.vector.tensor_copy(rstdq[:, pb], rst[:, :, 0:2])
            nc.vector.tensor_copy(m2q[:, pb], mu[:, :, 0:2])
            if not apply_gn:
                # k rstd -> exp scale table (0.125 * rstd_k)
                nc.vector.tensor_scalar(
                    rks[:, :, 2 * hp:2 * hp + 2], rst[:, :, 2:4], SCALE, 0.0,
                    op0=mybir.AluOpType.mult, op1=mybir.AluOpType.add)
            else:
                nc.vector.tensor_copy(rstdk[:, pb], rst[:, :, 2:4])
                nc.vector.tensor_copy(m2k[:, pb], mu[:, :, 2:4])

        def emit_pair_norms(hp):
            """q normalize into qnp0/qnp1 (+ for gn: full k LN in place)."""
            pb = hp % 2
            for tt in range(NT):
                for par in range(2):
                    dst = (qnp1[:, pb, tt, HD:2 * HD] if par
                           else qnp0[:, pb, tt, 0:HD])
                    nc.vector.tensor_scalar(
                        dst, q_ev[:, pb, tt, par * HD:(par + 1) * HD],
                        rstdq[:, pb, tt, par:par + 1],
                        m2q[:, pb, tt, par:par + 1],
                        op0=mybir.AluOpType.mult, op1=mybir.AluOpType.add)
                    if apply_gn:
                        nc.gpsimd.tensor_tensor(dst, dst, gamma_bc[:, 0:HD],
                                                op=mybir.AluOpType.mult)
                        nc.gpsimd.tensor_tensor(dst, dst, beta_bc[:, 0:HD],
                                                op=mybir.AluOpType.add)
                        kd = k_ev[:, pb, tt, par * HD:(par + 1) * HD]
                        nc.vector.tensor_scalar(
                            kd, kd,
                            rstdk[:, pb, tt, par:par + 1],
                            m2k[:, pb, tt, par:par + 1],
                            op0=mybir.AluOpType.mult, op1=mybir.AluOpType.add)
                        nc.gpsimd.tensor_tensor(kd, kd, gamma_bc[:, 0:HD],
                                                op=mybir.AluOpType.mult)
                        nc.gpsimd.tensor_tensor(kd, kd, beta_bc[:, 0:HD],
                                                op=mybir.AluOpType.add)

        def emit_pair_transposes(hp):
            # split each transpose into 2-block chunks so the descriptor
            # streams round-robin across DMA queues (a whole [P, 8x128]
            # transpose on one queue takes ~10us)
            pb = hp % 2
            kkT = qkpool.tile([P, N], BF16, tag="kkT", name="kkT", bufs=3)
            qp0 = qkpool.tile([P, N], BF16, tag="qp0", name="qp0", bufs=3)
            qp1 = qkpool.tile([P, N], BF16, tag="qp1", name="qp1", bufs=3)
            # 4-block chunks, issued first-needed-first (token-half major):
            # scores jt0 needs only the c=0 chunks of kkT/qp, so it unblocks
            # after 3 of the 6 ~1.1us SWDGE descriptor-gens, not all 6.
            for c in range(0, NT, 4):
                for dst, src in ((kkT, k_ev[:, pb]), (qp0, qnp0[:, pb]),
                                 (qp1, qnp1[:, pb])):
                    dr = dst.rearrange("p (b t) -> p b t", t=P)
                    nc.sync.dma_start_transpose(dr[:, c:c + 4], src[:, c:c + 4])
            return kkT, qp0, qp1

        def emit_normalize(h, pa0, pa1):
            for ic, pa in ((0, pa0), (1, pa1)):
                rcp_t = nrm.tile([HD, 512], F32, tag="rcp_t", name="rcp_t")
                s_sb = nrm.tile([HD, 512], F32, tag="s_sb", name="s_sb")
                nc.vector.tensor_copy(s_sb[:], pa[HD:2 * HD, :])
                nc.vector.reciprocal_approx_fast(rcp_t[:], s_sb[:])
                nc.vector.tensor_tensor(
                    attnoutT[(h % 2) * HD:(h % 2 + 1) * HD, h // 2,
                             ic * 512:(ic + 1) * 512],
                    pa[0:HD, :],
                    rcp_t[:],
                    op=mybir.AluOpType.mult,
                )

        def emit_head(h, kkT, qp0, qp1, prev, gsrc):
            """Scores+exp for head h, 1:1 interleaved with the attn@v of
            head h-1 (prev), plus one QKV group of pair gsrc per jt slot."""
            qT = qp0 if h % 2 == 0 else qp1
            E = epool.tile([P, NT, N], BF16, tag="E", name="E")
            if prev is not None:
                hprev, Eprev = prev
                pa0 = av_tile()
                pa1 = av_tile()
            for jt in range(NT):
                pst = st_tile()
                for ic in range(2):
                    nc.tensor.matmul(
                        pst[:, ic * 512:(ic + 1) * 512],
                        lhsT=kkT[:, jt * P:(jt + 1) * P],
                        rhs=qT[:, ic * 512:(ic + 1) * 512],
                        start=True,
                        stop=True,
                    )
                if apply_gn:
                    nc.scalar.activation(
                        E[:, jt, :], pst,
                        mybir.ActivationFunctionType.Exp, scale=SCALE)
                else:
                    nc.scalar.activation(
                        E[:, jt, :], pst,
                        mybir.ActivationFunctionType.Exp,
                        scale=rks[:, jt, h:h + 1])
                if prev is not None:
                    nc.tensor.matmul(
                        pa0, lhsT=vext[:, jt, hprev, :],
                        rhs=Eprev[:, jt, 0:512],
                        start=(jt == 0), stop=(jt == NT - 1),
                    )
                    nc.tensor.matmul(
                        pa1, lhsT=vext[:, jt, hprev, :],
                        rhs=Eprev[:, jt, 512:1024],
                        start=(jt == 0), stop=(jt == NT - 1),
                    )
                if gsrc is not None and jt % 2 == 1:
                    emit_group(gsrc, (h % 2) * 4 + jt // 2)
            if prev is not None:
                emit_normalize(hprev, pa0, pa1)
            return E

        def emit_av_tail(h, E):
            pa0 = av_tile()
            pa1 = av_tile()
            for jt in range(NT):
                nc.tensor.matmul(
                    pa0, lhsT=vext[:, jt, h, :], rhs=E[:, jt, 0:512],
                    start=(jt == 0), stop=(jt == NT - 1),
                )
                nc.tensor.matmul(
                    pa1, lhsT=vext[:, jt, h, :], rhs=E[:, jt, 512:1024],
                    start=(jt == 0), stop=(jt == NT - 1),
                )
            emit_normalize(h, pa0, pa1)

        # ---------------- the pipeline ----------------
        # prime: pairs 0 and 1
        tps = {}
        for tt in range(NT):
            emit_group(0, tt)
        emit_pair_stats(0)
        emit_pair_norms(0)
        tps[0] = emit_pair_transposes(0)
        for tt in range(NT):
            emit_group(1, tt)
        emit_pair_stats(1)
        emit_pair_norms(1)
        tps[1] = emit_pair_transposes(1)

        prev = None
        for h in range(H):
            hp, hh = divmod(h, 2)
            # pair hp+2's groups ride this pair's heads, 4 per head at odd jt
            gsrc = hp + 2 if hp + 2 < NP else None
            E = emit_head(h, *tps[hp], prev, gsrc)
            if hh == 1 and gsrc is not None:
                emit_pair_stats(gsrc)
                emit_pair_norms(gsrc)
                tps[gsrc] = emit_pair_transposes(gsrc)
                del tps[hp]
            prev = (h, E)
        emit_av_tail(*prev)

        # ---- output projection: outT[e, t] = projwT.T @ attnoutT ----
        for et in range(DC):
            ps0 = av_tile()
            ps1 = av_tile()
            for oc in range(DC):
                for half, pp in ((0, ps0), (1, ps1)):
                    nc.tensor.matmul(
                        pp,
                        lhsT=projwT[:, oc, et * P:(et + 1) * P],
                        rhs=attnoutT[:, oc, half * 512:(half + 1) * 512],
                        start=(oc == 0),
                        stop=(oc == DC - 1),
                    )
            for half, pp in ((0, ps0), (1, ps1)):
                ot = outp.tile([P, 512], F32, tag="outt", name="ot")
                nc.scalar.activation(
                    ot[:], pp, mybir.ActivationFunctionType.Identity,
                    bias=projb_col[:, et:et + 1],
                )
                # 2-way split: one 256KB f32 chunk on one queue is ~12us of
                # tail latency; SP is idle here (transposes are done)
                for c in range(2):
                    nc.sync.dma_start(
                        out_d[et * P:(et + 1) * P,
                              half * 512 + c * 256:half * 512 + (c + 1) * 256],
                        ot[:, c * 256:(c + 1) * 256],
                    )


_NC_CACHE = {}


def _get_nc(apply_gn=True):
    if apply_gn not in _NC_CACHE:
        _NC_CACHE[apply_gn] = _build_graph(apply_gn)
    return _NC_CACHE[apply_gn]


def make_in_maps(x, qkv_w, qkv_b, proj_w, proj_b, qn_gamma, qn_beta):
    """Host-side layout prep: transpose + bf16-cast x / weights; pack qkv
    weights and biases by head pair (q128|k128|v128 columns per pair)."""
    import ml_dtypes
    bf = ml_dtypes.bfloat16
    x = np.asarray(x, np.float32)
    qkv_w32 = np.asarray(qkv_w, np.float32)
    qkv_b32 = np.asarray(qkv_b, np.float32)
    wT = qkv_w32.T  # [D, 3D]: rows of qkv_w = out dims q|k|v
    pair_w = np.concatenate(
        [
            np.concatenate(
                [
                    wT[:, P * hp:P * (hp + 1)],
                    wT[:, D + P * hp:D + P * (hp + 1)],
                    wT[:, 2 * D + P * hp:2 * D + P * (hp + 1)],
                ],
                axis=1,
            )
            for hp in range(NP)
        ],
        axis=1,
    )  # [D, 6*384]
    bqk = np.concatenate(
        [
            np.concatenate(
                [qkv_b32[P * hp:P * (hp + 1)],
                 qkv_b32[D + P * hp:D + P * (hp + 1)]]
            )
            for hp in range(NP)
        ]
    )  # [6*256]
    bv = qkv_b32[2 * D:]  # [768] per-head v bias
    shared = {
        "qkv_wp": np.ascontiguousarray(pair_w.astype(bf)),
        "qkv_bqk": np.ascontiguousarray(bqk.astype(bf)),
        "qkv_bv": np.ascontiguousarray(bv.astype(bf)),
        "proj_w": np.ascontiguousarray(np.asarray(proj_w, np.float32).T.astype(bf)),
        "proj_b": np.ascontiguousarray(proj_b, np.float32),
        "qn_gamma": np.ascontiguousarray(qn_gamma, np.float32),
        "qn_beta": np.ascontiguousarray(qn_beta, np.float32),
    }
    return [
        {**shared, "x": np.ascontiguousarray(x[i].T.astype(bf))} for i in range(B)
    ]


def extract_output(res):
    return np.stack(
        [np.ascontiguousarray(res.results[i]["out"].T) for i in range(B)], axis=0
    )


def kernel(x, qkv_w, qkv_b, proj_w, proj_b, qn_gamma, qn_beta):
    qn_gamma = np.ascontiguousarray(qn_gamma, np.float32)
    qn_beta = np.ascontiguousarray(qn_beta, np.float32)
    apply_gn = not (np.all(qn_gamma == 1.0) and np.all(qn_beta == 0.0))
    nc = _get_nc(apply_gn)
    in_maps = make_in_maps(x, qkv_w, qkv_b, proj_w, proj_b, qn_gamma, qn_beta)
    res = run_bass_kernel_spmd(nc, in_maps, core_ids=list(range(B)))
    return extract_output(res)


# revision 33
# speedup vs baseline: 1.1737x; 1.0102x over previous
"""Multi-head attention forward on 8 TRN2 NeuronCores, data-parallel over batch.

Reference computation (per batch element b):
    qkv  = x @ qkv_w.T + qkv_b                     # [N, 3D]
    q, k = LN_headdim(q), LN_headdim(k)            # layernorm over head_dim=64
    S    = q @ k.T * hd^-0.5 ; A = softmax_j(S)    # per head
    out  = (A @ v) @ proj_w.T + proj_b             # [N, D]

v2 design (one batch element per core, no collectives), fully software-
pipelined so TensorE never waits for a phase boundary:
  - QKV is COLUMN-SLICED BY HEAD-PAIR: 6 pairs x 384 cols (q128|k128|v128,
    host-packed).  Pairs 0-1 prime the pipe; pair hp's 8 matmul groups ride
    inside head 2(hp-2)'s score stream, so the exp stream starts ~20us in
    and runs continuously to the end.  Group PSUM tiles ([P,384] f32, one
    bank) borrow the scores-tag rotation - PSUM stays within 8 banks
    (st [P,1024]x2 + av [P,512]x4).
  - k is NOT centered: against a fully-normalized q (sum_d q_n[d] = 0) the
    -mu_k term of k's layernorm vanishes in q_n.k; rstd_k folds into the
    exp's per-partition scale (scores^T has k-tokens on partitions), so k
    goes STRAIGHT from the QKV evacuation buffer into the block-transpose.
  - LN stats via one DVE bn_stats per (pair, tile); the per-pair rstd chain
    computes rstd = exp(-0.5 ln(var+eps)) on ScalarE - Ln/Exp/Identity all
    live in the SAME activation table as the softmax Exp, so the ACT table
    is never reloaded mid-stream (Sqrt would force a reload).
  - Scores computed TRANSPOSED: E = exp(scale_k * (k . q_n)) lands with
    k-tokens on partitions, directly the rhs of attn@v with V as lhsT.
  - q normalized into two zero-padded token-major buffers; plain 128x128
    block DMA transposes yield the K=128 zero-padded scores rhs.
  - Softmax denominators via 64 ones-columns in V (PSUM rows 64:128 hold
    the sums); normalize = SBUF copy + reciprocal_approx_fast + one VectorE
    multiply writing attnoutT.  (reciprocal_approx_fast must NOT read PSUM
    directly: it returns garbage on HW while passing in CoreSim.)
  - Heads software-pipelined 1:1 (scores of h interleave attn@v of h-1);
    projection computes outT = projwT.T @ attnoutT at the tail; ScalarE
    (idle after exps) applies the bias during PSUM evacuation; host flips.
"""

import os
import sys

import numpy as np

sys.path.insert(0, "/opt/trn_rl_repo")

from contextlib import ExitStack

import concourse.bass as bass
import concourse.tile as tile
from concourse import bacc, mybir
from concourse.bass_utils import run_bass_kernel_spmd

B, N, D = 8, 1024, 768
H, HD = 12, 64
NP = H // 2        # 6 head pairs
P = 128
NT = N // P        # 8 token tiles
DC = D // P        # 6 contraction subtiles
GC = 3 * P         # 384 qkv columns per pair group (q|k|v)
EPS = 1e-5
SCALE = HD ** -0.5  # 0.125
F32 = mybir.dt.float32
BF16 = mybir.dt.bfloat16


def _bcast_ap(ap_1d, parts):
    """View a 1-D DRAM AP as [parts, n] with partition stride 0 (broadcast)."""
    return bass.AP(
        tensor=ap_1d.tensor,
        offset=ap_1d.offset,
        ap=[[0, parts]] + list(ap_1d.ap),
    )


def _build_graph(apply_gn):
    nc = bacc.Bacc("TRN2", target_bir_lowering=False, debug=False, num_devices=B)

    x_d = nc.dram_tensor("x", [D, N], BF16, kind="ExternalInput").ap()
    # host-packed per-pair qkv weights: [D, pair*384] with 384 = q|k|v cols
    qkvw_d = nc.dram_tensor("qkv_wp", [D, NP * GC], BF16, kind="ExternalInput").ap()
    # host-packed biases: per-pair q|k (256 cols), per-head v (64)
    bqk_d = nc.dram_tensor("qkv_bqk", [NP * 2 * P], BF16, kind="ExternalInput").ap()
    bv_d = nc.dram_tensor("qkv_bv", [H * HD], BF16, kind="ExternalInput").ap()
    projw_d = nc.dram_tensor("proj_w", [D, D], BF16, kind="ExternalInput").ap()
    projb_d = nc.dram_tensor("proj_b", [D], F32, kind="ExternalInput").ap()
    gamma_d = nc.dram_tensor("qn_gamma", [HD], F32, kind="ExternalInput").ap()
    beta_d = nc.dram_tensor("qn_beta", [HD], F32, kind="ExternalInput").ap()
    # output is produced TRANSPOSED ([e, t]) in bf16; the host flips/casts
    out_d = nc.dram_tensor("out", [D, N], BF16, kind="ExternalOutput").ap()

    with tile.TileContext(nc) as tc:
        _emit(tc, out_d, x_d, qkvw_d, bqk_d, bv_d, projw_d, projb_d,
              gamma_d, beta_d, apply_gn)

    nc.compile()
    return nc


def _emit(tc, out_d, x_d, qkvw_d, bqk_d, bv_d, projw_d, projb_d,
          gamma_d, beta_d, apply_gn):
    nc = tc.nc
    ctx = ExitStack()
    with ctx:
        const = ctx.enter_context(tc.tile_pool(name="const", bufs=1))
        wpool = ctx.enter_context(tc.tile_pool(name="wts", bufs=1))
        data = ctx.enter_context(tc.tile_pool(name="data", bufs=1))
        epool = ctx.enter_context(tc.tile_pool(name="escore", bufs=2))
        qkpool = ctx.enter_context(tc.tile_pool(name="qk", bufs=2))
        spool = ctx.enter_context(tc.tile_pool(name="stats", bufs=2))
        outp = ctx.enter_context(tc.tile_pool(name="outp", bufs=3))
        nrm = ctx.enter_context(tc.tile_pool(name="nrm", bufs=2))
        ps = ctx.enter_context(tc.tile_pool(name="ps", bufs=1, space="PSUM"))

        # ---- weight / input DMAs, interleaved so pair-0 work can start
        # as early as possible: x[dc] + pair-0 weights first ----
        xT = wpool.tile([P, DC, N], BF16)            # [d_in, dc, t]
        qkvwT = wpool.tile([P, DC, NP, GC], BF16)    # [d_in, dc, pair, 384]
        projwT = wpool.tile([P, DC, D], BF16)        # [o_in, oc, e]
        x_r = x_d.rearrange("(dc p) t -> p dc t", p=P)
        w_r = qkvw_d.rearrange("(dc p) x -> p dc x", p=P)
        # Every dma_start costs ~0.6-1.2us of DGE time on its ISSUING
        # sequencer, so keep splits coarse and spread the latency-critical
        # issues across the engines that are idle at that point (SP + DVE +
        # ACT before the exp stream starts; GpSimd for everything
        # non-critical).
        # consumption order: group(0, tt0) reads token-half 0 of ALL dc plus
        # pair-0 weights, so land those 12 chunks first, then the rest
        for dc in range(DC):
            nc.sync.dma_start(xT[:, dc, 0:512], x_r[:, dc, 0:512])
            nc.scalar.dma_start(qkvwT[:, dc, 0, :], w_r[:, dc, 0:GC])
        for dc in range(DC):
            nc.scalar.dma_start(xT[:, dc, 512:1024], x_r[:, dc, 512:1024])
            nc.sync.dma_start(qkvwT[:, dc, 1, :], w_r[:, dc, GC:2 * GC])
        for hp in range(2, 4):
            for dc in range(DC):
                nc.gpsimd.dma_start(
                    qkvwT[:, dc, hp, :], w_r[:, dc, hp * GC:(hp + 1) * GC]
                )

        # broadcast constants (after the first-needed matmul operands)
        bqk_bc = const.tile([P, NP, 2 * P], BF16)
        nc.gpsimd.dma_start(bqk_bc[:], _bcast_ap(bqk_d, P))
        bv_bc = const.tile([P, H, HD], BF16)
        nc.gpsimd.dma_start(bv_bc[:], _bcast_ap(bv_d, P))
        projb_col = const.tile([P, DC], F32)
        nc.gpsimd.dma_start(projb_col[:], projb_d.rearrange("(et p) -> p et", p=P))
        if apply_gn:
            gamma_bc = const.tile([P, HD], F32)
            nc.gpsimd.dma_start(gamma_bc[:], _bcast_ap(gamma_d, P))
            beta_bc = const.tile([P, HD], F32)
            nc.gpsimd.dma_start(beta_bc[:], _bcast_ap(beta_d, P))

        for hp in range(4, NP):
            for dc in range(DC):
                nc.gpsimd.dma_start(
                    qkvwT[:, dc, hp, :], w_r[:, dc, hp * GC:(hp + 1) * GC]
                )
        for dc in range(DC):
            nc.gpsimd.dma_start(
                projwT[:, dc, :],
                projw_d.rearrange("(dc p) e -> p dc e", p=P)[:, dc, :],
            )

        # ---- persistent SBUF data tiles ----
        # qkv evacuations (per pair, double-buffered by pair parity); q and k
        # in separate tiles so the k block-transpose source is 2D-contiguous.
        # cols 0:64 = even head, 64:128 = odd head
        q_ev = data.tile([P, 2, NT, P], BF16)
        k_ev = data.tile([P, 2, NT, P], BF16)
        # q normalized, token-major, zero-padded halves (for DMA transpose)
        qnp0 = data.tile([P, 2, NT, P], BF16)   # cols 0:64 = q even head
        qnp1 = data.tile([P, 2, NT, P], BF16)   # cols 64:128 = q odd head
        # v with 64 ones-columns: attn@v psum rows 64:128 = softmax denoms.
        # memsets split per token-tile so the first QKV evacuations don't
        # serialize behind one long DVE memset.
        vext = data.tile([P, NT, H, 2 * HD], BF16)
        for tt in range(NT):
            nc.vector.memset(vext[:, tt, :, HD:2 * HD], 1.0)
        for pb in range(2):
            nc.vector.memset(qnp0[:, pb, :, HD:2 * HD], 0.0)
            nc.vector.memset(qnp1[:, pb, :, 0:HD], 0.0)
        # attnoutT [o_in, oc, t] written by the normalize step
        attnoutT = data.tile([P, DC, N], BF16)
        # 0.125 * rstd_k per (token-tile, head): per-partition exp scales
        rks = data.tile([P, NT, H], F32)
        # bn_stats output per pair: [P, parity, tt, 4 groups, 6]
        bnout = data.tile([P, 2, NT, 4, 6], F32)
        # per-pair q-norm params [P, parity, tt, grp] (0=q_even 1=q_odd)
        rstdq = data.tile([P, 2, NT, 2], F32)
        m2q = data.tile([P, 2, NT, 2], F32)
        if apply_gn:
            rstdk = data.tile([P, 2, NT, 2], F32)
            m2k = data.tile([P, 2, NT, 2], F32)

        # ---------------- emission helpers ----------------
        def st_tile():
            return ps.tile([P, N], F32, tag="st", name="ps_st", bufs=2)

        def av_tile():
            return ps.tile([P, 512], F32, tag="av", name="ps_av", bufs=4)

        def emit_group(hp, tt):
            """One QKV matmul group: psum[:, 0:384] = x_tt @ w_pair_hp,
            then evacuations (q|k to qk_ev, v to vext) and bn_stats."""
            pg = st_tile()
            for dc in range(DC):
                nc.tensor.matmul(
                    pg[:, 0:GC],
                    lhsT=xT[:, dc, tt * P:(tt + 1) * P],
                    rhs=qkvwT[:, dc, hp, :],
                    start=(dc == 0),
                    stop=(dc == DC - 1),
                )
            pb = hp % 2
            nc.vector.tensor_tensor(
                q_ev[:, pb, tt, :], pg[:, 0:P], bqk_bc[:, hp, 0:P],
                op=mybir.AluOpType.add,
            )
            nc.vector.tensor_tensor(
                k_ev[:, pb, tt, :], pg[:, P:2 * P], bqk_bc[:, hp, P:2 * P],
                op=mybir.AluOpType.add,
            )
            nc.vector.tensor_tensor(
                vext[:, tt, 2 * hp:2 * hp + 2, 0:HD],
                pg[:, 2 * P:3 * P].rearrange("p (s h) -> p s h", h=HD),
                bv_bc[:, 2 * hp:2 * hp + 2, :],
                op=mybir.AluOpType.add,
            )
            # HW restriction: one bn_stats = one 6-element output group
            for par in range(2):
                nc.vector.bn_stats(
                    bnout[:, pb, tt, par],
                    q_ev[:, pb, tt, par * HD:(par + 1) * HD],
                )
                nc.vector.bn_stats(
                    bnout[:, pb, tt, 2 + par],
                    k_ev[:, pb, tt, par * HD:(par + 1) * HD],
                )

        def emit_pair_stats(hp):
            """Per-pair rstd/m2 chain from bnout; fills rstdq/m2q/rks.
            bn_stats gives per group: [cnt_e, mean_e, M2_e, cnt_o, mean_o,
            M2_o] over even/odd elements.  mean = (me+mo)/2 and
            var = (M2e+M2o)/64 + ((me-mo)/2)^2."""
            pb = hp % 2
            me = bnout[:, pb, :, :, 1]    # [P, NT, 4]
            mo = bnout[:, pb, :, :, 4]
            M2e = bnout[:, pb, :, :, 2]
            M2o = bnout[:, pb, :, :, 5]
            a = spool.tile([P, NT, 4], F32, tag="a", name="sa")
            d = spool.tile([P, NT, 4], F32, tag="d", name="sd")
            var = spool.tile([P, NT, 4], F32, tag="var", name="svar")
            mu = spool.tile([P, NT, 4], F32, tag="mu", name="smu")
            rst = spool.tile([P, NT, 4], F32, tag="rst", name="srst")
            nc.vector.tensor_tensor(a, M2e, M2o, op=mybir.AluOpType.add)
            nc.vector.tensor_tensor(d, me, mo, op=mybir.AluOpType.subtract)
            nc.vector.tensor_tensor(d, d, d, op=mybir.AluOpType.mult)
            nc.vector.tensor_scalar(a, a, 1.0 / HD, EPS,
                                    op0=mybir.AluOpType.mult,
                                    op1=mybir.AluOpType.add)
            nc.vector.tensor_scalar(d, d, 0.25, 0.0,
                                    op0=mybir.AluOpType.mult,
                                    op1=mybir.AluOpType.add)
            nc.vector.tensor_tensor(var, a, d, op=mybir.AluOpType.add)
            # rstd = exp(-0.5 * ln(var+eps)); Ln/Exp share the act table
            nc.scalar.activation(a, var, mybir.ActivationFunctionType.Ln)
            nc.scalar.activation(rst, a, mybir.ActivationFunctionType.Exp,
                                 scale=-0.5)
            # m2 = -mean * rstd  (qnorm per-partition bias)
            nc.vector.tensor_tensor(mu, me, mo, op=mybir.AluOpType.add)
            nc.vector.tensor_tensor(mu, mu, rst, op=mybir.AluOpType.mult)
            nc.vector.tensor_scalar(mu, mu, -0.5, 0.0,
                                    op0=mybir.AluOpType.mult,
                                    op1=mybir.AluOpType.add)
            nc.vector.tensor_copy(rstdq[:, pb], rst[:, :, 0:2])
            nc.vector.tensor_copy(m2q[:, pb], mu[:, :, 0:2])
            if not apply_gn:
                # k rstd -> exp scale table (0.125 * rstd_k)
                nc.vector.tensor_scalar(
                    rks[:, :, 2 * hp:2 * hp + 2], rst[:, :, 2:4], SCALE, 0.0,
                    op0=mybir.AluOpType.mult, op1=mybir.AluOpType.add)
            else:
                nc.vector.tensor_copy(rstdk[:, pb], rst[:, :, 2:4])
                nc.vector.tensor_copy(m2k[:, pb], mu[:, :, 2:4])

        def emit_pair_norms(hp):
            """q normalize into qnp0/qnp1 (+ for gn: full k LN in place)."""
            pb = hp % 2
            for tt in range(NT):
                for par in range(2):
                    dst = (qnp1[:, pb, tt, HD:2 * HD] if par
                           else qnp0[:, pb, tt, 0:HD])
                    nc.vector.tensor_scalar(
                        dst, q_ev[:, pb, tt, par * HD:(par + 1) * HD],
                        rstdq[:, pb, tt, par:par + 1],
                        m2q[:, pb, tt, par:par + 1],
                        op0=mybir.AluOpType.mult, op1=mybir.AluOpType.add)
                    if apply_gn:
                        nc.gpsimd.tensor_tensor(dst, dst, gamma_bc[:, 0:HD],
                                                op=mybir.AluOpType.mult)
                        nc.gpsimd.tensor_tensor(dst, dst, beta_bc[:, 0:HD],
                                                op=mybir.AluOpType.add)
                        kd = k_ev[:, pb, tt, par * HD:(par + 1) * HD]
                        nc.vector.tensor_scalar(
                            kd, kd,
                            rstdk[:, pb, tt, par:par + 1],
                            m2k[:, pb, tt, par:par + 1],
                            op0=mybir.AluOpType.mult, op1=mybir.AluOpType.add)
                        nc.gpsimd.tensor_tensor(kd, kd, gamma_bc[:, 0:HD],
                                                op=mybir.AluOpType.mult)
                        nc.gpsimd.tensor_tensor(kd, kd, beta_bc[:, 0:HD],
                                                op=mybir.AluOpType.add)

        def emit_pair_transposes(hp):
            # split each transpose into 2-block chunks so the descriptor
            # streams round-robin across DMA queues (a whole [P, 8x128]
            # transpose on one queue takes ~10us)
            pb = hp % 2
            kkT = qkpool.tile([P, N], BF16, tag="kkT", name="kkT", bufs=3)
            qp0 = qkpool.tile([P, N], BF16, tag="qp0", name="qp0", bufs=3)
            qp1 = qkpool.tile([P, N], BF16, tag="qp1", name="qp1", bufs=3)
            # 4-block chunks, issued first-needed-first (token-half major):
            # scores jt0 needs only the c=0 chunks of kkT/qp, so it unblocks
            # after 3 of the 6 ~1.1us SWDGE descriptor-gens, not all 6.
            for c in range(0, NT, 4):
                for dst, src in ((kkT, k_ev[:, pb]), (qp0, qnp0[:, pb]),
                                 (qp1, qnp1[:, pb])):
                    dr = dst.rearrange("p (b t) -> p b t", t=P)
                    nc.sync.dma_start_transpose(dr[:, c:c + 4], src[:, c:c + 4])
            return kkT, qp0, qp1

        def emit_normalize(h, pa0, pa1):
            for ic, pa in ((0, pa0), (1, pa1)):
                rcp_t = nrm.tile([HD, 512], F32, tag="rcp_t", name="rcp_t")
                s_sb = nrm.tile([HD, 512], F32, tag="s_sb", name="s_sb")
                nc.vector.tensor_copy(s_sb[:], pa[HD:2 * HD, :])
                nc.vector.reciprocal_approx_fast(rcp_t[:], s_sb[:])
                nc.vector.tensor_tensor(
                    attnoutT[(h % 2) * HD:(h % 2 + 1) * HD, h // 2,
                             ic * 512:(ic + 1) * 512],
                    pa[0:HD, :],
                    rcp_t[:],
                    op=mybir.AluOpType.mult,
                )

        def emit_head(h, kkT, qp0, qp1, prev, gsrc):
            """Scores+exp for head h, 1:1 interleaved with the attn@v of
            head h-1 (prev), plus one QKV group of pair gsrc per jt slot."""
            qT = qp0 if h % 2 == 0 else qp1
            E = epool.tile([P, NT, N], BF16, tag="E", name="E")
            if prev is not None:
                hprev, Eprev = prev
                pa0 = av_tile()
                pa1 = av_tile()
            for jt in range(NT):
                pst = st_tile()
                for ic in range(2):
                    nc.tensor.matmul(
                        pst[:, ic * 512:(ic + 1) * 512],
                        lhsT=kkT[:, jt * P:(jt + 1) * P],
                        rhs=qT[:, ic * 512:(ic + 1) * 512],
                        start=True,
                        stop=True,
                    )
                if apply_gn:
                    nc.scalar.activation(
                        E[:, jt, :], pst,
                        mybir.ActivationFunctionType.Exp, scale=SCALE)
                else:
                    nc.scalar.activation(
                        E[:, jt, :], pst,
                        mybir.ActivationFunctionType.Exp,
                        scale=rks[:, jt, h:h + 1])
                if gsrc is not None and jt % 2 == 1:
                    emit_group(gsrc, (h % 2) * 4 + jt // 2)
            # attn@v of the previous head as two 8-matmul accumulation
            # chains: only the first matmul of each chain carries semaphore
            # waits, so the PE queue stays back-to-back and the stationary
            # loads overlap the streaming (interleaved per-jt emission costs
            # ~30% extra PE time in waits/unoverlapped loads)
            if prev is not None:
                for jt in range(NT):
                    nc.tensor.matmul(
                        pa0, lhsT=vext[:, jt, hprev, :],
                        rhs=Eprev[:, jt, 0:512],
                        start=(jt == 0), stop=(jt == NT - 1),
                    )
                for jt in range(NT):
                    nc.tensor.matmul(
                        pa1, lhsT=vext[:, jt, hprev, :],
                        rhs=Eprev[:, jt, 512:1024],
                        start=(jt == 0), stop=(jt == NT - 1),
                    )
                emit_normalize(hprev, pa0, pa1)
            return E

        def emit_av_tail(h, E):
            pa0 = av_tile()
            pa1 = av_tile()
            for jt in range(NT):
                nc.tensor.matmul(
                    pa0, lhsT=vext[:, jt, h, :], rhs=E[:, jt, 0:512],
                    start=(jt == 0), stop=(jt == NT - 1),
                )
            for jt in range(NT):
                nc.tensor.matmul(
                    pa1, lhsT=vext[:, jt, h, :], rhs=E[:, jt, 512:1024],
                    start=(jt == 0), stop=(jt == NT - 1),
                )
            emit_normalize(h, pa0, pa1)

        # ---------------- the pipeline ----------------
        # prime: pairs 0 and 1
        tps = {}
        for tt in range(NT):
            emit_group(0, tt)
        emit_pair_stats(0)
        emit_pair_norms(0)
        tps[0] = emit_pair_transposes(0)
        for tt in range(NT):
            emit_group(1, tt)
        emit_pair_stats(1)
        emit_pair_norms(1)
        tps[1] = emit_pair_transposes(1)

        prev = None
        for h in range(H):
            hp, hh = divmod(h, 2)
            # pair hp+2's groups ride this pair's heads, 4 per head at odd jt
            gsrc = hp + 2 if hp + 2 < NP else None
            E = emit_head(h, *tps[hp], prev, gsrc)
            if hh == 1 and gsrc is not None:
                emit_pair_stats(gsrc)
                emit_pair_norms(gsrc)
                tps[gsrc] = emit_pair_transposes(gsrc)
                del tps[hp]
            prev = (h, E)
        emit_av_tail(*prev)

        # ---- output projection: outT[e, t] = projwT.T @ attnoutT ----
        for et in range(DC):
            ps0 = av_tile()
            ps1 = av_tile()
            for half, pp in ((0, ps0), (1, ps1)):
                for oc in range(DC):
                    nc.tensor.matmul(
                        pp,
                        lhsT=projwT[:, oc, et * P:(et + 1) * P],
                        rhs=attnoutT[:, oc, half * 512:(half + 1) * 512],
                        start=(oc == 0),
                        stop=(oc == DC - 1),
                    )
            for half, pp in ((0, ps0), (1, ps1)):
                ot = outp.tile([P, 512], BF16, tag="outt", name="ot")
                nc.scalar.activation(
                    ot[:], pp, mybir.ActivationFunctionType.Identity,
                    bias=projb_col[:, et:et + 1],
                )
                # 2-way split: one 256KB f32 chunk on one queue is ~12us of
                # tail latency; SP is idle here (transposes are done)
                for c in range(2):
                    nc.sync.dma_start(
                        out_d[et * P:(et + 1) * P,
                              half * 512 + c * 256:half * 512 + (c + 1) * 256],
                        ot[:, c * 256:(c + 1) * 256],
                    )


_NC_CACHE = {}


def _get_nc(apply_gn=True):
    if apply_gn not in _NC_CACHE:
        _NC_CACHE[apply_gn] = _build_graph(apply_gn)
    return _NC_CACHE[apply_gn]


def make_in_maps(x, qkv_w, qkv_b, proj_w, proj_b, qn_gamma, qn_beta):
    """Host-side layout prep: transpose + bf16-cast x / weights; pack qkv
    weights and biases by head pair (q128|k128|v128 columns per pair)."""
    import ml_dtypes
    bf = ml_dtypes.bfloat16
    x = np.asarray(x, np.float32)
    qkv_w32 = np.asarray(qkv_w, np.float32)
    qkv_b32 = np.asarray(qkv_b, np.float32)
    wT = qkv_w32.T  # [D, 3D]: rows of qkv_w = out dims q|k|v
    pair_w = np.concatenate(
        [
            np.concatenate(
                [
                    wT[:, P * hp:P * (hp + 1)],
                    wT[:, D + P * hp:D + P * (hp + 1)],
                    wT[:, 2 * D + P * hp:2 * D + P * (hp + 1)],
                ],
                axis=1,
            )
            for hp in range(NP)
        ],
        axis=1,
    )  # [D, 6*384]
    bqk = np.concatenate(
        [
            np.concatenate(
                [qkv_b32[P * hp:P * (hp + 1)],
                 qkv_b32[D + P * hp:D + P * (hp + 1)]]
            )
            for hp in range(NP)
        ]
    )  # [6*256]
    bv = qkv_b32[2 * D:]  # [768] per-head v bias
    shared = {
        "qkv_wp": np.ascontiguousarray(pair_w.astype(bf)),
        "qkv_bqk": np.ascontiguousarray(bqk.astype(bf)),
        "qkv_bv": np.ascontiguousarray(bv.astype(bf)),
        "proj_w": np.ascontiguousarray(np.asarray(proj_w, np.float32).T.astype(bf)),
        "proj_b": np.ascontiguousarray(proj_b, np.float32),
        "qn_gamma": np.ascontiguousarray(qn_gamma, np.float32),
        "qn_beta": np.ascontiguousarray(qn_beta, np.float32),
    }
    return [
        {**shared, "x": np.ascontiguousarray(x[i].T.astype(bf))} for i in range(B)
    ]


def extract_output(res):
    return np.stack(
        [np.ascontiguousarray(res.results[i]["out"].T.astype(np.float32))
         for i in range(B)], axis=0
    )


def kernel(x, qkv_w, qkv_b, proj_w, proj_b, qn_gamma, qn_beta):
    qn_gamma = np.ascontiguousarray(qn_gamma, np.float32)
    qn_beta = np.ascontiguousarray(qn_beta, np.float32)
    apply_gn = not (np.all(qn_gamma == 1.0) and np.all(qn_beta == 0.0))
    nc = _get_nc(apply_gn)
    in_maps = make_in_maps(x, qkv_w, qkv_b, proj_w, proj_b, qn_gamma, qn_beta)
    res = run_bass_kernel_spmd(nc, in_maps, core_ids=list(range(B)))
    return extract_output(res)


# revision 37
# speedup vs baseline: 1.3446x; 1.1455x over previous
"""Multi-head attention forward on 8 TRN2 NeuronCores, data-parallel over batch.

Reference computation (per batch element b):
    qkv  = x @ qkv_w.T + qkv_b                     # [N, 3D]
    q, k = LN_headdim(q), LN_headdim(k)            # layernorm over head_dim=64
    S    = q @ k.T * hd^-0.5 ; A = softmax_j(S)    # per head
    out  = (A @ v) @ proj_w.T + proj_b             # [N, D]

v2 design (one batch element per core, no collectives), fully software-
pipelined so TensorE never waits for a phase boundary:
  - QKV is COLUMN-SLICED BY HEAD-PAIR: 6 pairs x 384 cols (q128|k128|v128,
    host-packed).  Pairs 0-1 prime the pipe; pair hp's 8 matmul groups ride
    inside head 2(hp-2)'s score stream, so the exp stream starts ~20us in
    and runs continuously to the end.  Group PSUM tiles ([P,384] f32, one
    bank) borrow the scores-tag rotation - PSUM stays within 8 banks
    (st [P,1024]x2 + av [P,512]x4).
  - k is NOT centered: against a fully-normalized q (sum_d q_n[d] = 0) the
    -mu_k term of k's layernorm vanishes in q_n.k; rstd_k folds into the
    exp's per-partition scale (scores^T has k-tokens on partitions), so k
    goes STRAIGHT from the QKV evacuation buffer into the block-transpose.
  - LN stats via one DVE bn_stats per (pair, tile); the per-pair rstd chain
    computes rstd = exp(-0.5 ln(var+eps)) on ScalarE - Ln/Exp/Identity all
    live in the SAME activation table as the softmax Exp, so the ACT table
    is never reloaded mid-stream (Sqrt would force a reload).
  - Scores computed TRANSPOSED: E = exp(scale_k * (k . q_n)) lands with
    k-tokens on partitions, directly the rhs of attn@v with V as lhsT.
  - q normalized into two zero-padded token-major buffers; plain 128x128
    block DMA transposes yield the K=128 zero-padded scores rhs.
  - Softmax denominators via 64 ones-columns in V (PSUM rows 64:128 hold
    the sums); normalize = SBUF copy + reciprocal_approx_fast + one VectorE
    multiply writing attnoutT.  (reciprocal_approx_fast must NOT read PSUM
    directly: it returns garbage on HW while passing in CoreSim.)
  - Heads software-pipelined 1:1 (scores of h interleave attn@v of h-1);
    projection computes outT = projwT.T @ attnoutT at the tail; ScalarE
    (idle after exps) applies the bias during PSUM evacuation; host flips.
"""

import os
import sys

import numpy as np

sys.path.insert(0, "/opt/trn_rl_repo")

from contextlib import ExitStack

import concourse.bass as bass
import concourse.tile as tile
from concourse import bacc, mybir
from concourse.bass_utils import run_bass_kernel_spmd

B, N, D = 8, 1024, 768
H, HD = 12, 64
NP = H // 2        # 6 head pairs
P = 128
NT = N // P        # 8 token tiles
DC = D // P        # 6 contraction subtiles
GC = 3 * P         # 384 qkv columns per pair group (q|k|v)
EPS = 1e-5
SCALE = HD ** -0.5  # 0.125
F32 = mybir.dt.float32
BF16 = mybir.dt.bfloat16


def _bcast_ap(ap_1d, parts):
    """View a 1-D DRAM AP as [parts, n] with partition stride 0 (broadcast)."""
    return bass.AP(
        tensor=ap_1d.tensor,
        offset=ap_1d.offset,
        ap=[[0, parts]] + list(ap_1d.ap),
    )


def _build_graph(apply_gn):
    nc = bacc.Bacc("TRN2", target_bir_lowering=False, debug=False, num_devices=B)

    x_d = nc.dram_tensor("x", [D, N], BF16, kind="ExternalInput").ap()
    # host-packed per-pair qkv weights: [D, pair*384] with 384 = q|k|v cols
    qkvw_d = nc.dram_tensor("qkv_wp", [D, NP * GC], BF16, kind="ExternalInput").ap()
    # host-packed biases: per-pair q|k (256 cols), per-head v (64)
    bqk_d = nc.dram_tensor("qkv_bqk", [NP * 2 * P], BF16, kind="ExternalInput").ap()
    bv_d = nc.dram_tensor("qkv_bv", [H * HD], BF16, kind="ExternalInput").ap()
    projw_d = nc.dram_tensor("proj_w", [D, D], BF16, kind="ExternalInput").ap()
    projb_d = nc.dram_tensor("proj_b", [D], F32, kind="ExternalInput").ap()
    gamma_d = nc.dram_tensor("qn_gamma", [HD], F32, kind="ExternalInput").ap()
    beta_d = nc.dram_tensor("qn_beta", [HD], F32, kind="ExternalInput").ap()
    # output is produced TRANSPOSED ([e, t]) in bf16; the host flips/casts
    out_d = nc.dram_tensor("out", [D, N], BF16, kind="ExternalOutput").ap()

    with tile.TileContext(nc) as tc:
        _emit(tc, out_d, x_d, qkvw_d, bqk_d, bv_d, projw_d, projb_d,
              gamma_d, beta_d, apply_gn)

    nc.compile()
    return nc


def _emit(tc, out_d, x_d, qkvw_d, bqk_d, bv_d, projw_d, projb_d,
          gamma_d, beta_d, apply_gn):
    nc = tc.nc
    ctx = ExitStack()
    with ctx:
        const = ctx.enter_context(tc.tile_pool(name="const", bufs=1))
        wpool = ctx.enter_context(tc.tile_pool(name="wts", bufs=1))
        data = ctx.enter_context(tc.tile_pool(name="data", bufs=1))
        epool = ctx.enter_context(tc.tile_pool(name="escore", bufs=2))
        qkpool = ctx.enter_context(tc.tile_pool(name="qk", bufs=2))
        spool = ctx.enter_context(tc.tile_pool(name="stats", bufs=2))
        outp = ctx.enter_context(tc.tile_pool(name="outp", bufs=3))
        nrm = ctx.enter_context(tc.tile_pool(name="nrm", bufs=2))
        ps = ctx.enter_context(tc.tile_pool(name="ps", bufs=1, space="PSUM"))

        # ---- weight / input DMAs, interleaved so pair-0 work can start
        # as early as possible: x[dc] + pair-0 weights first ----
        xT = wpool.tile([P, DC, N], BF16)            # [d_in, dc, t]
        qkvwT = wpool.tile([P, DC, NP, GC], BF16)    # [d_in, dc, pair, 384]
        projwT = wpool.tile([P, DC, D], BF16)        # [o_in, oc, e]
        x_r = x_d.rearrange("(dc p) t -> p dc t", p=P)
        w_r = qkvw_d.rearrange("(dc p) x -> p dc x", p=P)
        # Every dma_start costs ~0.6-1.2us of DGE time on its ISSUING
        # sequencer, so keep splits coarse and spread the latency-critical
        # issues across the engines that are idle at that point (SP + DVE +
        # ACT before the exp stream starts; GpSimd for everything
        # non-critical).
        # consumption order: group(0, tt0) reads token-half 0 of ALL dc plus
        # pair-0 weights, so land those chunks first, finely split across
        # queues (HWDGE issues are cheap; SWDGE transposes are not)
        for dc in range(DC):
            nc.sync.dma_start(xT[:, dc, 0:256], x_r[:, dc, 0:256])
            nc.sync.dma_start(xT[:, dc, 256:512], x_r[:, dc, 256:512])
            nc.scalar.dma_start(qkvwT[:, dc, 0, 0:192], w_r[:, dc, 0:192])
            nc.scalar.dma_start(qkvwT[:, dc, 0, 192:GC], w_r[:, dc, 192:GC])
        for dc in range(DC):
            nc.scalar.dma_start(xT[:, dc, 512:1024], x_r[:, dc, 512:1024])
            nc.sync.dma_start(qkvwT[:, dc, 1, :], w_r[:, dc, GC:2 * GC])
        for hp in range(2, 4):
            for dc in range(DC):
                nc.gpsimd.dma_start(
                    qkvwT[:, dc, hp, :], w_r[:, dc, hp * GC:(hp + 1) * GC]
                )

        # broadcast constants (after the first-needed matmul operands)
        bqk_bc = const.tile([P, NP, 2 * P], BF16)
        nc.gpsimd.dma_start(bqk_bc[:], _bcast_ap(bqk_d, P))
        bv_bc = const.tile([P, H, HD], BF16)
        nc.gpsimd.dma_start(bv_bc[:], _bcast_ap(bv_d, P))
        projb_col = const.tile([P, DC], F32)
        nc.gpsimd.dma_start(projb_col[:], projb_d.rearrange("(et p) -> p et", p=P))
        if apply_gn:
            gamma_bc = const.tile([P, HD], F32)
            nc.gpsimd.dma_start(gamma_bc[:], _bcast_ap(gamma_d, P))
            beta_bc = const.tile([P, HD], F32)
            nc.gpsimd.dma_start(beta_bc[:], _bcast_ap(beta_d, P))

        for hp in range(4, NP):
            for dc in range(DC):
                nc.gpsimd.dma_start(
                    qkvwT[:, dc, hp, :], w_r[:, dc, hp * GC:(hp + 1) * GC]
                )
        for dc in range(DC):
            nc.gpsimd.dma_start(
                projwT[:, dc, :],
                projw_d.rearrange("(dc p) e -> p dc e", p=P)[:, dc, :],
            )

        # ---- persistent SBUF data tiles ----
        # qkv evacuations (per pair, double-buffered by pair parity); q and k
        # in separate tiles so the k block-transpose source is 2D-contiguous.
        # cols 0:64 = even head, 64:128 = odd head
        q_ev = data.tile([P, 2, NT, P], BF16)
        k_ev = data.tile([P, 2, NT, P], BF16)
        # q normalized, token-major, zero-padded halves (for DMA transpose)
        qnp0 = data.tile([P, 2, NT, P], BF16)   # cols 0:64 = q even head
        qnp1 = data.tile([P, 2, NT, P], BF16)   # cols 64:128 = q odd head
        # v with 64 ones-columns: attn@v psum rows 64:128 = softmax denoms.
        # memsets are split per token-tile and interleaved into the priming
        # loop below so the first QKV evacuations don't serialize behind
        # ~9us of up-front DVE memset work.
        vext = data.tile([P, NT, H, 2 * HD], BF16)
        # attnoutT [o_in, oc, t] written by the normalize step
        attnoutT = data.tile([P, DC, N], BF16)
        # 0.125 * rstd_k per (token-tile, head): per-partition exp scales
        rks = data.tile([P, NT, H], F32)
        # bn_stats output per pair: [P, parity, tt, 4 groups, 6]
        bnout = data.tile([P, 2, NT, 4, 6], F32)
        # per-pair q-norm params [P, parity, tt, grp] (0=q_even 1=q_odd)
        rstdq = data.tile([P, 2, NT, 2], F32)
        m2q = data.tile([P, 2, NT, 2], F32)
        if apply_gn:
            rstdk = data.tile([P, 2, NT, 2], F32)
            m2k = data.tile([P, 2, NT, 2], F32)

        # ---------------- emission helpers ----------------
        def st_tile():
            return ps.tile([P, N], F32, tag="st", name="ps_st", bufs=2)

        def av_tile():
            return ps.tile([P, 512], F32, tag="av", name="ps_av", bufs=4)

        def emit_group(hp, tt):
            """One QKV matmul group: psum[:, 0:384] = x_tt @ w_pair_hp,
            then evacuations (q|k to qk_ev, v to vext) and bn_stats.
            Uses the av psum tag: its rotation slack (group evacs drain fast,
            normalize lags a full head) means no stall, whereas sharing the
            scores tag paced groups by the exp stream."""
            pg = av_tile()
            for dc in range(DC):
                nc.tensor.matmul(
                    pg[:, 0:GC],
                    lhsT=xT[:, dc, tt * P:(tt + 1) * P],
                    rhs=qkvwT[:, dc, hp, :],
                    start=(dc == 0),
                    stop=(dc == DC - 1),
                )
            pb = hp % 2
            nc.vector.tensor_tensor(
                q_ev[:, pb, tt, :], pg[:, 0:P], bqk_bc[:, hp, 0:P],
                op=mybir.AluOpType.add,
            )
            nc.vector.tensor_tensor(
                k_ev[:, pb, tt, :], pg[:, P:2 * P], bqk_bc[:, hp, P:2 * P],
                op=mybir.AluOpType.add,
            )
            nc.vector.tensor_tensor(
                vext[:, tt, 2 * hp:2 * hp + 2, 0:HD],
                pg[:, 2 * P:3 * P].rearrange("p (s h) -> p s h", h=HD),
                bv_bc[:, 2 * hp:2 * hp + 2, :],
                op=mybir.AluOpType.add,
            )
            # HW restriction: one bn_stats = one 6-element output group
            for par in range(2):
                nc.vector.bn_stats(
                    bnout[:, pb, tt, par],
                    q_ev[:, pb, tt, par * HD:(par + 1) * HD],
                )
                nc.vector.bn_stats(
                    bnout[:, pb, tt, 2 + par],
                    k_ev[:, pb, tt, par * HD:(par + 1) * HD],
                )

        def emit_pair_stats(hp):
            """Per-pair rstd/m2 chain from bnout; fills rstdq/m2q/rks.
            bn_stats gives per group: [cnt_e, mean_e, M2_e, cnt_o, mean_o,
            M2_o] over even/odd elements.  mean = (me+mo)/2 and
            var = (M2e+M2o)/64 + ((me-mo)/2)^2."""
            pb = hp % 2
            me = bnout[:, pb, :, :, 1]    # [P, NT, 4]
            mo = bnout[:, pb, :, :, 4]
            M2e = bnout[:, pb, :, :, 2]
            M2o = bnout[:, pb, :, :, 5]
            a = spool.tile([P, NT, 4], F32, tag="a", name="sa")
            d = spool.tile([P, NT, 4], F32, tag="d", name="sd")
            var = spool.tile([P, NT, 4], F32, tag="var", name="svar")
            mu = spool.tile([P, NT, 4], F32, tag="mu", name="smu")
            rst = spool.tile([P, NT, 4], F32, tag="rst", name="srst")
            nc.vector.tensor_tensor(a, M2e, M2o, op=mybir.AluOpType.add)
            nc.vector.tensor_tensor(d, me, mo, op=mybir.AluOpType.subtract)
            nc.vector.tensor_tensor(d, d, d, op=mybir.AluOpType.mult)
            nc.vector.tensor_scalar(a, a, 1.0 / HD, EPS,
                                    op0=mybir.AluOpType.mult,
                                    op1=mybir.AluOpType.add)
            nc.vector.tensor_scalar(d, d, 0.25, 0.0,
                                    op0=mybir.AluOpType.mult,
                                    op1=mybir.AluOpType.add)
            nc.vector.tensor_tensor(var, a, d, op=mybir.AluOpType.add)
            # rstd = exp(-0.5 * ln(var+eps)); Ln/Exp share the act table
            nc.scalar.activation(a, var, mybir.ActivationFunctionType.Ln)
            nc.scalar.activation(rst, a, mybir.ActivationFunctionType.Exp,
                                 scale=-0.5)
            # m2 = -mean * rstd  (qnorm per-partition bias)
            nc.vector.tensor_tensor(mu, me, mo, op=mybir.AluOpType.add)
            nc.vector.tensor_tensor(mu, mu, rst, op=mybir.AluOpType.mult)
            nc.vector.tensor_scalar(mu, mu, -0.5, 0.0,
                                    op0=mybir.AluOpType.mult,
                                    op1=mybir.AluOpType.add)
            nc.vector.tensor_copy(rstdq[:, pb], rst[:, :, 0:2])
            nc.vector.tensor_copy(m2q[:, pb], mu[:, :, 0:2])
            if not apply_gn:
                # k rstd -> exp scale table (0.125 * rstd_k)
                nc.vector.tensor_scalar(
                    rks[:, :, 2 * hp:2 * hp + 2], rst[:, :, 2:4], SCALE, 0.0,
                    op0=mybir.AluOpType.mult, op1=mybir.AluOpType.add)
            else:
                nc.vector.tensor_copy(rstdk[:, pb], rst[:, :, 2:4])
                nc.vector.tensor_copy(m2k[:, pb], mu[:, :, 2:4])

        def emit_pair_norms(hp):
            """q normalize into qnp0/qnp1 (+ for gn: full k LN in place)."""
            pb = hp % 2
            for tt in range(NT):
                for par in range(2):
                    dst = (qnp1[:, pb, tt, HD:2 * HD] if par
                           else qnp0[:, pb, tt, 0:HD])
                    nc.vector.tensor_scalar(
                        dst, q_ev[:, pb, tt, par * HD:(par + 1) * HD],
                        rstdq[:, pb, tt, par:par + 1],
                        m2q[:, pb, tt, par:par + 1],
                        op0=mybir.AluOpType.mult, op1=mybir.AluOpType.add)
                    if apply_gn:
                        nc.gpsimd.tensor_tensor(dst, dst, gamma_bc[:, 0:HD],
                                                op=mybir.AluOpType.mult)
                        nc.gpsimd.tensor_tensor(dst, dst, beta_bc[:, 0:HD],
                                                op=mybir.AluOpType.add)
                        kd = k_ev[:, pb, tt, par * HD:(par + 1) * HD]
                        nc.vector.tensor_scalar(
                            kd, kd,
                            rstdk[:, pb, tt, par:par + 1],
                            m2k[:, pb, tt, par:par + 1],
                            op0=mybir.AluOpType.mult, op1=mybir.AluOpType.add)
                        nc.gpsimd.tensor_tensor(kd, kd, gamma_bc[:, 0:HD],
                                                op=mybir.AluOpType.mult)
                        nc.gpsimd.tensor_tensor(kd, kd, beta_bc[:, 0:HD],
                                                op=mybir.AluOpType.add)

        def emit_pair_transposes(hp):
            # split each transpose into 2-block chunks so the descriptor
            # streams round-robin across DMA queues (a whole [P, 8x128]
            # transpose on one queue takes ~10us)
            pb = hp % 2
            kkT = qkpool.tile([P, N], BF16, tag="kkT", name="kkT", bufs=3)
            qp0 = qkpool.tile([P, N], BF16, tag="qp0", name="qp0", bufs=3)
            qp1 = qkpool.tile([P, N], BF16, tag="qp1", name="qp1", bufs=3)
            # 4-block chunks, issued first-needed-first (token-half major):
            # scores jt0 needs only the c=0 chunks of kkT/qp, so it unblocks
            # after 3 of the 6 ~1.1us SWDGE descriptor-gens, not all 6.
            for c in range(0, NT, 4):
                for dst, src in ((kkT, k_ev[:, pb]), (qp0, qnp0[:, pb]),
                                 (qp1, qnp1[:, pb])):
                    dr = dst.rearrange("p (b t) -> p b t", t=P)
                    nc.sync.dma_start_transpose(dr[:, c:c + 4], src[:, c:c + 4])
            return kkT, qp0, qp1

        def emit_normalize(h, pa0, pa1):
            for ic, pa in ((0, pa0), (1, pa1)):
                rcp_t = nrm.tile([HD, 512], F32, tag="rcp_t", name="rcp_t")
                s_sb = nrm.tile([HD, 512], F32, tag="s_sb", name="s_sb")
                nc.vector.tensor_copy(s_sb[:], pa[HD:2 * HD, :])
                nc.vector.reciprocal_approx_fast(rcp_t[:], s_sb[:])
                nc.vector.tensor_tensor(
                    attnoutT[(h % 2) * HD:(h % 2 + 1) * HD, h // 2,
                             ic * 512:(ic + 1) * 512],
                    pa[0:HD, :],
                    rcp_t[:],
                    op=mybir.AluOpType.mult,
                )

        def emit_head(h, kkT, qp0, qp1, prev, gsrc):
            """Scores+exp for head h, 1:1 interleaved with the attn@v of
            head h-1 (prev), plus one QKV group of pair gsrc per jt slot."""
            qT = qp0 if h % 2 == 0 else qp1
            E = epool.tile([P, NT, N], BF16, tag="E", name="E")
            if prev is not None:
                hprev, Eprev = prev
                pa0 = av_tile()
                pa1 = av_tile()
            for jt in range(NT):
                pst = st_tile()
                for ic in range(2):
                    nc.tensor.matmul(
                        pst[:, ic * 512:(ic + 1) * 512],
                        lhsT=kkT[:, jt * P:(jt + 1) * P],
                        rhs=qT[:, ic * 512:(ic + 1) * 512],
                        start=True,
                        stop=True,
                    )
                if apply_gn:
                    nc.scalar.activation(
                        E[:, jt, :], pst,
                        mybir.ActivationFunctionType.Exp, scale=SCALE)
                else:
                    nc.scalar.activation(
                        E[:, jt, :], pst,
                        mybir.ActivationFunctionType.Exp,
                        scale=rks[:, jt, h:h + 1])
                if gsrc is not None and jt % 2 == 1:
                    emit_group(gsrc, (h % 2) * 4 + jt // 2)
            # attn@v of the previous head as two 8-matmul accumulation
            # chains: only the first matmul of each chain carries semaphore
            # waits, so the PE queue stays back-to-back and the stationary
            # loads overlap the streaming (interleaved per-jt emission costs
            # ~30% extra PE time in waits/unoverlapped loads)
            if prev is not None:
                for jt in range(NT):
                    nc.tensor.matmul(
                        pa0, lhsT=vext[:, jt, hprev, :],
                        rhs=Eprev[:, jt, 0:512],
                        start=(jt == 0), stop=(jt == NT - 1),
                    )
                for jt in range(NT):
                    nc.tensor.matmul(
                        pa1, lhsT=vext[:, jt, hprev, :],
                        rhs=Eprev[:, jt, 512:1024],
                        start=(jt == 0), stop=(jt == NT - 1),
                    )
                emit_normalize(hprev, pa0, pa1)
            return E

        def emit_av_tail(h, E):
            pa0 = av_tile()
            pa1 = av_tile()
            for jt in range(NT):
                nc.tensor.matmul(
                    pa0, lhsT=vext[:, jt, h, :], rhs=E[:, jt, 0:512],
                    start=(jt == 0), stop=(jt == NT - 1),
                )
            for jt in range(NT):
                nc.tensor.matmul(
                    pa1, lhsT=vext[:, jt, h, :], rhs=E[:, jt, 512:1024],
                    start=(jt == 0), stop=(jt == NT - 1),
                )
            emit_normalize(h, pa0, pa1)

        # ---------------- the pipeline ----------------
        # prime: pairs 0 and 1 (memsets ride between the groups' DVE work)
        tps = {}
        for tt in range(NT):
            nc.vector.memset(vext[:, tt, :, HD:2 * HD], 1.0)
            emit_group(0, tt)
            if tt == 4:
                nc.vector.memset(qnp0[:, 0, :, HD:2 * HD], 0.0)
            if tt == 5:
                nc.vector.memset(qnp1[:, 0, :, 0:HD], 0.0)
        emit_pair_stats(0)
        emit_pair_norms(0)
        tps[0] = emit_pair_transposes(0)
        for tt in range(NT):
            emit_group(1, tt)
            if tt == 0:
                nc.vector.memset(qnp0[:, 1, :, HD:2 * HD], 0.0)
            if tt == 1:
                nc.vector.memset(qnp1[:, 1, :, 0:HD], 0.0)
        emit_pair_stats(1)
        emit_pair_norms(1)
        tps[1] = emit_pair_transposes(1)

        prev = None
        for h in range(H):
            hp, hh = divmod(h, 2)
            # pair hp+2's groups ride this pair's heads, 4 per head at odd jt
            gsrc = hp + 2 if hp + 2 < NP else None
            E = emit_head(h, *tps[hp], prev, gsrc)
            if hh == 1 and gsrc is not None:
                emit_pair_stats(gsrc)
                emit_pair_norms(gsrc)
                tps[gsrc] = emit_pair_transposes(gsrc)
                del tps[hp]
            prev = (h, E)
        emit_av_tail(*prev)

        # ---- output projection: outT[e, t] = projwT.T @ attnoutT ----
        for et in range(DC):
            ps0 = av_tile()
            ps1 = av_tile()
            for half, pp in ((0, ps0), (1, ps1)):
                for oc in range(DC):
                    nc.tensor.matmul(
                        pp,
                        lhsT=projwT[:, oc, et * P:(et + 1) * P],
                        rhs=attnoutT[:, oc, half * 512:(half + 1) * 512],
                        start=(oc == 0),
                        stop=(oc == DC - 1),
                    )
            for half, pp in ((0, ps0), (1, ps1)):
                ot = outp.tile([P, 512], BF16, tag="outt", name="ot")
                nc.scalar.activation(
                    ot[:], pp, mybir.ActivationFunctionType.Identity,
                    bias=projb_col[:, et:et + 1],
                )
                # 2-way split: one 256KB f32 chunk on one queue is ~12us of
                # tail latency; SP is idle here (transposes are done)
                for c in range(2):
                    nc.sync.dma_start(
                        out_d[et * P:(et + 1) * P,
                              half * 512 + c * 256:half * 512 + (c + 1) * 256],
                        ot[:, c * 256:(c + 1) * 256],
                    )


_NC_CACHE = {}


def _get_nc(apply_gn=True):
    if apply_gn not in _NC_CACHE:
        _NC_CACHE[apply_gn] = _build_graph(apply_gn)
    return _NC_CACHE[apply_gn]


def make_in_maps(x, qkv_w, qkv_b, proj_w, proj_b, qn_gamma, qn_beta):
    """Host-side layout prep: transpose + bf16-cast x / weights; pack qkv
    weights and biases by head pair (q128|k128|v128 columns per pair)."""
    import ml_dtypes
    bf = ml_dtypes.bfloat16
    x = np.asarray(x, np.float32)
    qkv_w32 = np.asarray(qkv_w, np.float32)
    qkv_b32 = np.asarray(qkv_b, np.float32)
    wT = qkv_w32.T  # [D, 3D]: rows of qkv_w = out dims q|k|v
    pair_w = np.concatenate(
        [
            np.concatenate(
                [
                    wT[:, P * hp:P * (hp + 1)],
                    wT[:, D + P * hp:D + P * (hp + 1)],
                    wT[:, 2 * D + P * hp:2 * D + P * (hp + 1)],
                ],
                axis=1,
            )
            for hp in range(NP)
        ],
        axis=1,
    )  # [D, 6*384]
    bqk = np.concatenate(
        [
            np.concatenate(
                [qkv_b32[P * hp:P * (hp + 1)],
                 qkv_b32[D + P * hp:D + P * (hp + 1)]]
            )
            for hp in range(NP)
        ]
    )  # [6*256]
    bv = qkv_b32[2 * D:]  # [768] per-head v bias
    shared = {
        "qkv_wp": np.ascontiguousarray(pair_w.astype(bf)),
        "qkv_bqk": np.ascontiguousarray(bqk.astype(bf)),
        "qkv_bv": np.ascontiguousarray(bv.astype(bf)),
        "proj_w": np.ascontiguousarray(np.asarray(proj_w, np.float32).T.astype(bf)),
        "proj_b": np.ascontiguousarray(proj_b, np.float32),
        "qn_gamma": np.ascontiguousarray(qn_gamma, np.float32),
        "qn_beta": np.ascontiguousarray(qn_beta, np.float32),
    }
    return [
        {**shared, "x": np.ascontiguousarray(x[i].T.astype(bf))} for i in range(B)
    ]


def extract_output(res):
    return np.stack(
        [np.ascontiguousarray(res.results[i]["out"].T.astype(np.float32))
         for i in range(B)], axis=0
    )


def kernel(x, qkv_w, qkv_b, proj_w, proj_b, qn_gamma, qn_beta):
    qn_gamma = np.ascontiguousarray(qn_gamma, np.float32)
    qn_beta = np.ascontiguousarray(qn_beta, np.float32)
    apply_gn = not (np.all(qn_gamma == 1.0) and np.all(qn_beta == 0.0))
    nc = _get_nc(apply_gn)
    in_maps = make_in_maps(x, qkv_w, qkv_b, proj_w, proj_b, qn_gamma, qn_beta)
    res = run_bass_kernel_spmd(nc, in_maps, core_ids=list(range(B)))
    return extract_output(res)
